# revision 3
# baseline (speedup 1.0000x reference)
"""Cross-attention Bass/Tile kernel for Trainium2, sharded over 8 NeuronCores.

Problem (fixed shapes): B=2, T=2048, C=1024, H=16 heads, D=64.
    q = x_q @ Wq + bq;  kv = x_kv @ Wkv + bkv;  k, v = split(kv)
    y = softmax(q k^T / sqrt(D)) v;  out = y @ Wo + bo
Sharding: 8 cores = 2 (batch) x 4 (head groups of 4 heads, 256 channels).

Fully bf16 dataflow (host casts x and weights; rel-err budget 2e-2 >> bf16
noise; PSUM accumulation stays fp32 except the single-shot S logits which
land in bf16 PSUM to halve bank usage).  Attention AV uses exp(S) as the
matmul *stationary* ([tk,128] x [tk,65] -> [tq,65]) so each product streams
65 moving columns instead of 512 (AV: 131k -> 67k PE cycles).  The softmax
denominator rides as a ones-column of V and lands per-partition; the
normalization is a DVE reciprocal + per-partition scalar multiply.
Normalized y transposes back to y^T with bf16 PE transposes.

Schedule: the two tq=0 attention passes are woven *into* phase A (K/V
prep) so the Activation engine's exp stream starts ~10us in; Q-prep for
tq+1 and the deferred output projection weave into the attention passes
as PE filler.  PSUM: 2 x [128,1024 bf16] S slots (2 banks), 4 x
[128,4,65 f32] y accumulators (4), 2 x 2KB weave slots (2).
"""

import numpy as np

B = 2
T = 2048
C = 1024
H = 16
D = 64
NCORES = 8
TPG = 4  # tensor-parallel group size (head groups)
HL = H // TPG  # heads per core = 4
CL = HL * D  # local channels = 256
P = 128

_CACHE = {}


def _build():
    import concourse.tile as tile
    from concourse import bacc, mybir
    from concourse.masks import make_identity

    f32 = mybir.dt.float32
    bf16 = mybir.dt.bfloat16
    Exp = mybir.ActivationFunctionType.Exp

    nc = bacc.Bacc("TRN2", target_bir_lowering=False, debug=False)

    xq_d = nc.dram_tensor("xq", [T, C], bf16, kind="ExternalInput")
    xkv_d = nc.dram_tensor("xkv", [T, C], bf16, kind="ExternalInput")
    wq_d = nc.dram_tensor("wq", [C, CL], bf16, kind="ExternalInput")
    wk_d = nc.dram_tensor("wk", [C, CL], bf16, kind="ExternalInput")
    wv_d = nc.dram_tensor("wv", [C, CL], bf16, kind="ExternalInput")
    wo_d = nc.dram_tensor("wo", [CL, C], bf16, kind="ExternalInput")
    bq_d = nc.dram_tensor("bq", [CL], f32, kind="ExternalInput")
    bk_d = nc.dram_tensor("bk", [CL], f32, kind="ExternalInput")
    out_d = nc.dram_tensor("out", [T, C], f32, kind="ExternalOutput")

    KC = C // P  # 8 contraction chunks for the projections
    NT = T // P  # 16 token chunks of 128
    NQ = 4  # tq chunks of 512
    QW = T // NQ  # 512
    DC = CL // P  # 2 chunks of d_local
    LAG = 5

    with tile.TileContext(nc) as tc:
        with (
            tc.tile_pool(name="const", bufs=1) as const,
            tc.tile_pool(name="persist", bufs=1) as persist,
            tc.tile_pool(name="xnat", bufs=4) as xnat,
            tc.tile_pool(name="xt", bufs=1) as xtp,
            tc.tile_pool(name="ework", bufs=54) as ework,
            tc.tile_pool(name="norm2", bufs=2) as norm2,
            tc.tile_pool(name="outst", bufs=6) as outst,
        ):
            # ---- constants / weights (weights via SWDGE, one DMA per
            # tensor, first-consumer first, so HWDGE is free for x loads
            # and nothing stalls on trickled weight chunks) ----
            ident = const.tile([P, P], f32)
            make_identity(nc, ident)
            identb = const.tile([P, P], bf16)
            nc.vector.tensor_copy(identb, ident)
            ones4_f32 = const.tile([P, HL, 1], f32)
            nc.vector.memset(ones4_f32, 1.0)

            wq_sb = const.tile([P, KC, CL], bf16)
            wk_sb = const.tile([P, KC, CL], bf16)
            wv_sb = const.tile([P, KC, CL], bf16)
            wo_sb = const.tile([P, DC, C], bf16)
            for w_sb, w_d in ((wv_sb, wv_d), (wq_sb, wq_d), (wk_sb, wk_d)):
                nc.gpsimd.dma_start(
                    w_sb, w_d.rearrange("(o p) d -> p o d", p=P)
                )
            bq_sb = const.tile([P, DC], f32)
            bk_sb = const.tile([P, DC], f32)

            # ---- persistent activations ----
            qt_sb = persist.tile([P, DC, T], bf16)  # Q^T  [d, t]
            kt_sb = persist.tile([P, DC, T], bf16)  # K^T  [d, t]
            v_sb = persist.tile([P, NT, HL, 66], bf16)  # V|1 [t, h, d+1]
            yt_sb = persist.tile([P, DC, T], bf16)  # y^T  [d, t] (normalized)

            # ---- kernel-wide PSUM ----
            ps_s = tc.alloc_tile_pool(name="ps_s", bufs=2, space="PSUM")
            ps_acc = tc.alloc_tile_pool(name="ps_acc", bufs=1, space="PSUM")
            ps_y = tc.alloc_tile_pool(name="ps_y", bufs=2, space="PSUM")

            # ---------- emission helpers ----------
            def q_prep_units(tq):
                """Work units (thunks) producing xq^T and Q^T for `tq`."""
                xq_t = xtp.tile([P, KC, QW], bf16, tag="xqT", name="xq_t")
                units = []
                trs = []
                state = {}
                for ts_ in range(4):
                    tch = tq * 4 + ts_

                    def dma_u(ts_=ts_, tch=tch):
                        x_nat = xnat.tile([P, C], bf16, tag="xq_nat", name="x_nat")
                        state[ts_] = x_nat
                        nc.sync.dma_start(x_nat, xq_d[tch * P : (tch + 1) * P, :])

                    units.append(dma_u)
                    for grp in range(2):

                        def tr_u(ts_=ts_, grp=grp):
                            x_nat = state[ts_]
                            tp = ps_y.tile([P, 4 * P], bf16, tag="y", name="tp")
                            for cc in range(4):
                                c = grp * 4 + cc
                                nc.tensor.transpose(
                                    tp[:, cc * P : (cc + 1) * P],
                                    x_nat[:, c * P : (c + 1) * P],
                                    identb,
                                )
                            nc.vector.tensor_copy(
                                xq_t[
                                    :, grp * 4 : (grp + 1) * 4, ts_ * P : (ts_ + 1) * P
                                ],
                                tp.rearrange("p (c t) -> p c t", c=4),
                            )

                        trs.append(tr_u)
                units.extend(trs)  # all 4 DMAs go out before any PE work
                for dc in range(DC):

                    def proj_u(dc=dc):
                        pp = ps_y.tile([P, QW], f32, tag="y", name="pp")
                        for c in range(KC):
                            nc.tensor.matmul(
                                pp,
                                wq_sb[:, c, dc * P : (dc + 1) * P],
                                xq_t[:, c, :],
                                start=(c == 0),
                                stop=(c == KC - 1),
                            )
                        nc.vector.tensor_scalar_add(
                            qt_sb[:, dc, tq * QW : (tq + 1) * QW],
                            pp,
                            bq_sb[:, dc : dc + 1],
                        )

                    units.append(proj_u)
                return units

            def po_units(tq, on_act=False):
                """Output-projection work units for `tq` (yt must be final)."""
                units = []
                for ts_ in range(4):
                    tch = tq * 4 + ts_
                    for co in range(2):

                        def u(tch=tch, co=co):
                            po = ps_y.tile([P, QW], f32, tag="y", name="po")
                            for dc in range(DC):
                                nc.tensor.matmul(
                                    po,
                                    yt_sb[:, dc, tch * P : (tch + 1) * P],
                                    wo_sb[:, dc, co * QW : (co + 1) * QW],
                                    start=(dc == 0),
                                    stop=(dc == DC - 1),
                                )
                            o_st = outst.tile([P, QW], f32, tag="o")
                            if on_act and (ts_ + co) % 2 == 0:
                                nc.scalar.copy(o_st, po)
                            else:
                                nc.vector.tensor_copy(o_st, po)
                            # final batch alternates HWDGE/SWDGE so the two
                            # descriptor generators overlap in the tail
                            dma_q = nc.sync
                            dma_q.dma_start(
                                out_d[
                                    tch * P : (tch + 1) * P, co * QW : (co + 1) * QW
                                ],
                                o_st,
                            )

                        units.append(u)
                return units

            # ---------- attention streaming machinery ----------
            y_tiles = {}
            e_tiles = {}
            yq = []  # FIFO of (k, hc, tk) awaiting their AV matmuls
            unit_q = []  # (tag, thunk) PE filler work units
            s1c = [0, 0]  # held S(1,hc) stream cursors during phase A

            def emit_sexp(k, hc, tk):
                sp = ps_s.tile([P, 2 * QW], f32, tag="s", name="sp")
                for hh in range(2):
                    nc.tensor.matmul(
                        sp[:, hh * QW : (hh + 1) * QW],
                        kt_sb[hh * 64 : (hh + 1) * 64, hc, tk * P : (tk + 1) * P],
                        qt_sb[hh * 64 : (hh + 1) * 64, hc, k * QW : (k + 1) * QW],
                        start=True,
                        stop=True,
                        tile_position=(hh * 64, 0),
                    )
                e2 = ework.tile([P, 2 * QW], bf16, tag="e", name="e2")
                nc.scalar.activation(e2, sp, Exp, scale=0.125)
                e_tiles[(k, hc, tk)] = e2

            def emit_y(k, hc, tk):
                """AV partials: exp(S) chunk as stationary, V|1 as moving."""
                if (k, hc) not in y_tiles:
                    y_tiles[(k, hc)] = [
                        ps_acc.tile([P, 4, 65], f32, tag=f"acc{i}", name=f"y_ps{i}")
                        for i in range(2)
                    ]
                y_pair = y_tiles[(k, hc)]
                e2 = e_tiles.pop((k, hc, tk))
                for hh in range(2):
                    h = 2 * hc + hh
                    for cq in range(4):
                        # one accumulation group per PSUM bank (= per hh
                        # tile): start zeroes the whole 2KB zero-region, so
                        # only the very first matmul into the bank starts
                        # and only the very last stops
                        nc.tensor.matmul(
                            y_pair[hh][:, cq, :],
                            e2[:, hh * QW + cq * P : hh * QW + (cq + 1) * P],
                            v_sb[:, tk, h, 0:65],
                            start=(tk == 0 and cq == 0),
                            stop=(tk == NT - 1 and cq == 3),
                        )

            def emit_norm(k, hc, fuse_po=False):
                """Normalize by the ridden-along denominator; build y^T.
                With fuse_po (final pass), each 128-token chunk's output
                projection is emitted the moment its y^T slice lands."""
                y_pair = y_tiles.pop((k, hc))
                den = norm2.tile([P, 2, 4], f32, tag="den")
                for hh in range(2):
                    nc.vector.tensor_copy(den[:, hh, :], y_pair[hh][:, :, 64])
                rec = norm2.tile([P, 2, 4], f32, tag="rec")
                with nc.allow_low_precision(reason="softmax denom reciprocal"):
                    nc.vector.reciprocal(rec, den)
                y2 = norm2.tile([P, 4, P], bf16, tag="y2")
                for hh in range(2):
                    for cq in range(4):
                        nc.vector.tensor_scalar_mul(
                            y2[:, cq, hh * 64 : (hh + 1) * 64],
                            y_pair[hh][:, cq, 0:64],
                            rec[:, hh, cq : cq + 1],
                        )
                tp_y = ps_y.tile([P, 4, P], bf16, tag="y", name="tp_y")
                for cq in range(4):
                    nc.tensor.transpose(tp_y[:, cq, :], y2[:, cq, :], identb)
                nc.vector.tensor_copy(
                    yt_sb[:, hc, k * QW : (k + 1) * QW],
                    tp_y.rearrange("p c t -> p (c t)"),
                )
                if hc == DC - 1 and k < NQ - 1:
                    unit_q.extend(("po", u) for u in po_units(k))

            def attn_step(k, hc, tk, hold=False, lag=LAG):
                """Stream one S/exp step.  hold=True defers the AV work
                entirely (retired later via push_retirement units)."""
                emit_sexp(k, hc, tk)
                if not hold:
                    yq.append((k, hc, tk))
                    if len(yq) > lag:
                        emit_y(*yq.pop(0))

            def push_retirement(k, hc):
                """Queue a held pass's AV matmuls + normalization as filler
                units; they pop during the next pass's S stream."""
                for tk in range(NT):
                    unit_q.append(
                        ("ret", lambda k=k, hc=hc, tk=tk: emit_y(k, hc, tk))
                    )
                unit_q.append(("ret", lambda k=k, hc=hc: emit_norm(k, hc)))

            def pop_units(n):
                for _ in range(n):
                    if unit_q:
                        unit_q.pop(0)[1]()

            # ---- phase A: K^T and V prep with the two tq=0 attention
            # passes woven in (exp starts as soon as the first K half and
            # Q^T(tq0) exist) ----
            # first two x_kv chunk DMAs go out before the x_q block so the
            # PE has transpose work at t=0
            kv_pre = []
            for i in range(2):
                t = xnat.tile([P, C], bf16, tag="xkv_nat", name="kv_nat")
                nc.sync.dma_start(t, xkv_d[i * P : (i + 1) * P, :])
                kv_pre.append(t)
            qunits = q_prep_units(0)
            for u in qunits[:4]:  # xq DMAs up front
                u()
            qrest = qunits[4:]
            # remaining weights after the x loads: their DMA transfers
            # yield the shared DMA engines to the latency-critical x_q path
            nc.gpsimd.dma_start(bq_sb, bq_d.rearrange("(o p) -> p o", p=P))
            nc.gpsimd.dma_start(bk_sb, bk_d.rearrange("(o p) -> p o", p=P))
            nc.gpsimd.dma_start(wo_sb, wo_d.rearrange("(o p) n -> p o n", p=P))
            for tq in range(NQ):
                xkv_t = xtp.tile([P, KC, QW], bf16, tag="xkvT", name="xkv_t")
                for ts_ in range(4):
                    tch = tq * 4 + ts_
                    if tch < len(kv_pre):
                        kv_nat = kv_pre[tch]
                    else:
                        kv_nat = xnat.tile(
                            [P, C], bf16, tag="xkv_nat", name="kv_nat"
                        )
                        nc.sync.dma_start(
                            kv_nat, xkv_d[tch * P : (tch + 1) * P, :]
                        )
                    tp = ps_y.tile([P, KC * P], bf16, tag="y", name="tp8")
                    for c in range(KC):
                        nc.tensor.transpose(
                            tp[:, c * P : (c + 1) * P],
                            kv_nat[:, c * P : (c + 1) * P],
                            identb,
                        )
                    nc.vector.tensor_copy(
                        xkv_t[:, :, ts_ * P : (ts_ + 1) * P],
                        tp.rearrange("p (c t) -> p c t", c=KC),
                    )
                    # finish Q-prep(0) before the first K half completes
                    while qrest and ts_ % 2 == 0:
                        qrest.pop(0)()
                    # queue Q-prep(1) early; it must finish before the held
                    # S(1,*) streams start at tq==2
                    if tq == 0 and ts_ == 0:
                        unit_q.extend(("qp1", u) for u in q_prep_units(1))
                    # K projection first on odd chunks (its DVE bias-add
                    # gates the S matmuls; V's copies can wait)
                    if ts_ in (1, 3):
                        half = ts_ // 2
                        hsl = slice(half * 256, (half + 1) * 256)
                        for dc in range(DC):
                            pp = ps_y.tile([P, 256], f32, tag="y", name="ppk")
                            for c in range(KC):
                                nc.tensor.matmul(
                                    pp,
                                    wk_sb[:, c, dc * P : (dc + 1) * P],
                                    xkv_t[:, c, hsl],
                                    start=(c == 0),
                                    stop=(c == KC - 1),
                                )
                            nc.vector.tensor_scalar_add(
                                kt_sb[:, dc, tq * QW + half * 256 : tq * QW + (half + 1) * 256],
                                pp,
                                bk_sb[:, dc : dc + 1],
                            )
                    # V projection for this token chunk
                    pv = ps_y.tile([P, CL], f32, tag="y", name="pv")
                    for c in range(KC):
                        nc.tensor.matmul(
                            pv,
                            xkv_t[:, c, ts_ * P : (ts_ + 1) * P],
                            wv_sb[:, c, :],
                            start=(c == 0),
                            stop=(c == KC - 1),
                        )
                    nc.vector.tensor_copy(
                        v_sb[:, tch, :, 0:64],
                        pv.rearrange("p (h d) -> p h d", h=HL),
                    )
                    nc.vector.tensor_copy(v_sb[:, tch, :, 64:65], ones4_f32)
                    pop_units(2 if ts_ % 2 == 0 else 1)
                    if tq == 2 and ts_ == 1:
                        # correctness: Q^T(1) writes must be emitted before
                        # the held S(1,*) streams read them
                        while any(t == "qp1" for t, _ in unit_q):
                            pop_units(1)
                    if ts_ in (1, 3):
                        half = ts_ // 2
                        # attention on the two freshly available tk chunks.
                        # Pass (0,0) runs with live (lagged) AV; (0,1) and,
                        # once Q^T(1) exists, (1,0)/(1,1) stream S/exp held
                        # so the ACT exp pipeline is dense through phase A.
                        for tkn in (
                            tq * 4 + half * 2,
                            tq * 4 + half * 2 + 1,
                        ):
                            attn_step(0, 0, tkn)
                        for tkn in (
                            tq * 4 + half * 2,
                            tq * 4 + half * 2 + 1,
                        ):
                            attn_step(0, 1, tkn, hold=True)
                        if tq >= 2:
                            for hcx in range(DC):
                                for _ in range(2):
                                    if s1c[hcx] < NT - 8:
                                        attn_step(1, hcx, s1c[hcx], hold=True)
                                        s1c[hcx] += 1

            # finish the held S(1,*) streams, retire the phase-A passes
            unit_q.extend(("qp2", u) for u in q_prep_units(2))
            for hcx in range(DC):
                for tkx in range(s1c[hcx], NT):
                    attn_step(1, hcx, tkx, hold=True)
                    pop_units(1)
            while yq:
                emit_y(*yq.pop(0))
            emit_norm(0, 0)
            push_retirement(0, 1)
            push_retirement(1, 0)
            push_retirement(1, 1)

            # ---- phase B: remaining S/exp streams; AV+norm of each pass
            # retires as filler during the next pass's stream ----
            for k, hc in [(2, 0), (2, 1), (3, 0), (3, 1)]:
                last = k == NQ - 1 and hc == DC - 1
                if hc == 0:
                    # Q-prep(k) must be fully emitted before S reads Q^T(k)
                    while any(t == f"qp{k}" for t, _ in unit_q):
                        pop_units(1)
                    if k + 1 < NQ:
                        unit_q.extend(
                            (f"qp{k + 1}", u) for u in q_prep_units(k + 1)
                        )
                for tk in range(NT):
                    attn_step(k, hc, tk, hold=True)
                    pop_units(3 if len(unit_q) > 26 else 2)
                if not last:
                    push_retirement(k, hc)
            while unit_q:
                pop_units(1)
            for tk in range(NT):
                emit_y(NQ - 1, DC - 1, tk)
            emit_norm(NQ - 1, DC - 1)
            for u in po_units(NQ - 1, on_act=True):
                u()
            while unit_q:
                pop_units(1)

            ps_y.release()
            ps_acc.release()
            ps_s.release()

    nc.compile()
    return nc


def _get_nc():
    if "nc" not in _CACHE:
        _CACHE["nc"] = _build()
    return _CACHE["nc"]


def _shard_inputs(x_q, x_kv, Wq, bq, Wkv, bkv, Wo):
    import ml_dtypes

    bf = ml_dtypes.bfloat16
    in_maps = []
    for core in range(NCORES):
        b = core // TPG
        g = core % TPG
        cols = slice(g * CL, (g + 1) * CL)
        in_maps.append(
            {
                "xq": np.ascontiguousarray(x_q[b].astype(bf)),
                "xkv": np.ascontiguousarray(x_kv[b].astype(bf)),
                "wq": np.ascontiguousarray(Wq[:, cols].astype(bf)),
                "wk": np.ascontiguousarray(Wkv[:, :C][:, cols].astype(bf)),
                "wv": np.ascontiguousarray(Wkv[:, C:][:, cols].astype(bf)),
                "wo": np.ascontiguousarray(Wo[g * CL : (g + 1) * CL, :].astype(bf)),
                "bq": np.ascontiguousarray(bq[cols]),
                "bk": np.ascontiguousarray(bkv[:C][cols]),
            }
        )
    return in_maps


def kernel(x_q, x_kv, Wq, bq, Wkv, bkv, Wo, bo):
    from concourse.bass_utils import run_bass_kernel_spmd

    x_q = np.asarray(x_q, dtype=np.float32)
    x_kv = np.asarray(x_kv, dtype=np.float32)
    Wq = np.asarray(Wq, dtype=np.float32)
    bq = np.asarray(bq, dtype=np.float32)
    Wkv = np.asarray(Wkv, dtype=np.float32)
    bkv = np.asarray(bkv, dtype=np.float32)
    Wo = np.asarray(Wo, dtype=np.float32)
    bo = np.asarray(bo, dtype=np.float32)

    nc = _get_nc()
    in_maps = _shard_inputs(x_q, x_kv, Wq, bq, Wkv, bkv, Wo)

    res = run_bass_kernel_spmd(nc, in_maps, core_ids=list(range(NCORES)))

    # host-side gather: sum tensor-parallel partials; add exact bias terms
    bias_full = bkv[C:] @ Wo + bo  # v-bias through Wo, plus output bias
    out = np.zeros((B, T, C), dtype=np.float32)
    for core in range(NCORES):
        out[core // TPG] += res.results[core]["out"]
    out += bias_full[None, None, :]
    return out


# revision 4
# speedup vs baseline: 1.0010x; 1.0010x over previous
"""Cross-attention Bass/Tile kernel for Trainium2, sharded over 8 NeuronCores.

Problem (fixed shapes): B=2, T=2048, C=1024, H=16 heads, D=64.
    q = x_q @ Wq + bq;  kv = x_kv @ Wkv + bkv;  k, v = split(kv)
    y = softmax(q k^T / sqrt(D)) v;  out = y @ Wo + bo
Sharding: 8 cores = 2 (batch) x 4 (head groups of 4 heads, 256 channels).

Fully bf16 dataflow (host casts x and weights; rel-err budget 2e-2 >> bf16
noise; PSUM accumulation stays fp32 except the single-shot S logits which
land in bf16 PSUM to halve bank usage).  Attention AV uses exp(S) as the
matmul *stationary* ([tk,128] x [tk,65] -> [tq,65]) so each product streams
65 moving columns instead of 512 (AV: 131k -> 67k PE cycles).  The softmax
denominator rides as a ones-column of V and lands per-partition; the
normalization is a DVE reciprocal + per-partition scalar multiply.
Normalized y transposes back to y^T with bf16 PE transposes.

Schedule: the two tq=0 attention passes are woven *into* phase A (K/V
prep) so the Activation engine's exp stream starts ~10us in; Q-prep for
tq+1 and the deferred output projection weave into the attention passes
as PE filler.  PSUM: 2 x [128,1024 bf16] S slots (2 banks), 4 x
[128,4,65 f32] y accumulators (4), 2 x 2KB weave slots (2).
"""

import numpy as np

B = 2
T = 2048
C = 1024
H = 16
D = 64
NCORES = 8
TPG = 4  # tensor-parallel group size (head groups)
HL = H // TPG  # heads per core = 4
CL = HL * D  # local channels = 256
P = 128

_CACHE = {}


def _build():
    import concourse.tile as tile
    from concourse import bacc, mybir
    from concourse.masks import make_identity

    f32 = mybir.dt.float32
    bf16 = mybir.dt.bfloat16
    Exp = mybir.ActivationFunctionType.Exp

    nc = bacc.Bacc("TRN2", target_bir_lowering=False, debug=False)

    xq_d = nc.dram_tensor("xq", [T, C], bf16, kind="ExternalInput")
    xkv_d = nc.dram_tensor("xkv", [T, C], bf16, kind="ExternalInput")
    wq_d = nc.dram_tensor("wq", [C, CL], bf16, kind="ExternalInput")
    wk_d = nc.dram_tensor("wk", [C, CL], bf16, kind="ExternalInput")
    wv_d = nc.dram_tensor("wv", [C, CL], bf16, kind="ExternalInput")
    wo_d = nc.dram_tensor("wo", [CL, C], bf16, kind="ExternalInput")
    bq_d = nc.dram_tensor("bq", [CL], f32, kind="ExternalInput")
    bk_d = nc.dram_tensor("bk", [CL], f32, kind="ExternalInput")
    out_d = nc.dram_tensor("out", [T, C], f32, kind="ExternalOutput")

    KC = C // P  # 8 contraction chunks for the projections
    NT = T // P  # 16 token chunks of 128
    NQ = 4  # tq chunks of 512
    QW = T // NQ  # 512
    DC = CL // P  # 2 chunks of d_local
    LAG = 5

    with tile.TileContext(nc) as tc:
        with (
            tc.tile_pool(name="const", bufs=1) as const,
            tc.tile_pool(name="persist", bufs=1) as persist,
            tc.tile_pool(name="xnat", bufs=4) as xnat,
            tc.tile_pool(name="xt", bufs=1) as xtp,
            tc.tile_pool(name="ework", bufs=54) as ework,
            tc.tile_pool(name="norm2", bufs=2) as norm2,
            tc.tile_pool(name="outst", bufs=6) as outst,
        ):
            # ---- constants / weights (weights via SWDGE, one DMA per
            # tensor, first-consumer first, so HWDGE is free for x loads
            # and nothing stalls on trickled weight chunks) ----
            ident = const.tile([P, P], f32)
            make_identity(nc, ident)
            identb = const.tile([P, P], bf16)
            nc.vector.tensor_copy(identb, ident)
            ones4_f32 = const.tile([P, HL, 1], f32)
            nc.vector.memset(ones4_f32, 1.0)

            wq_sb = const.tile([P, KC, CL], bf16)
            wk_sb = const.tile([P, KC, CL], bf16)
            wv_sb = const.tile([P, KC, CL], bf16)
            wo_sb = const.tile([P, DC, C], bf16)
            for w_sb, w_d in ((wv_sb, wv_d), (wq_sb, wq_d), (wk_sb, wk_d)):
                nc.gpsimd.dma_start(
                    w_sb, w_d.rearrange("(o p) d -> p o d", p=P)
                )
            bq_sb = const.tile([P, DC], f32)
            bk_sb = const.tile([P, DC], f32)

            # ---- persistent activations ----
            qt_sb = persist.tile([P, DC, T], bf16)  # Q^T  [d, t]
            kt_sb = persist.tile([P, DC, T], bf16)  # K^T  [d, t]
            v_sb = persist.tile([P, NT, HL, 66], bf16)  # V|1 [t, h, d+1]
            yt_sb = persist.tile([P, DC, T], bf16)  # y^T  [d, t] (normalized)

            # ---- kernel-wide PSUM ----
            ps_s = tc.alloc_tile_pool(name="ps_s", bufs=2, space="PSUM")
            ps_acc = tc.alloc_tile_pool(name="ps_acc", bufs=1, space="PSUM")
            ps_y = tc.alloc_tile_pool(name="ps_y", bufs=2, space="PSUM")

            # ---------- emission helpers ----------
            def q_prep_units(tq):
                """Work units (thunks) producing xq^T and Q^T for `tq`."""
                xq_t = xtp.tile([P, KC, QW], bf16, tag="xqT", name="xq_t")
                units = []
                trs = []
                state = {}
                for ts_ in range(4):
                    tch = tq * 4 + ts_

                    def dma_u(ts_=ts_, tch=tch):
                        x_nat = xnat.tile([P, C], bf16, tag="xq_nat", name="x_nat")
                        state[ts_] = x_nat
                        nc.sync.dma_start(x_nat, xq_d[tch * P : (tch + 1) * P, :])

                    units.append(dma_u)
                    for grp in range(2):

                        def tr_u(ts_=ts_, grp=grp):
                            x_nat = state[ts_]
                            tp = ps_y.tile([P, 4 * P], bf16, tag="y", name="tp")
                            for cc in range(4):
                                c = grp * 4 + cc
                                nc.tensor.transpose(
                                    tp[:, cc * P : (cc + 1) * P],
                                    x_nat[:, c * P : (c + 1) * P],
                                    identb,
                                )
                            nc.vector.tensor_copy(
                                xq_t[
                                    :, grp * 4 : (grp + 1) * 4, ts_ * P : (ts_ + 1) * P
                                ],
                                tp.rearrange("p (c t) -> p c t", c=4),
                            )

                        trs.append(tr_u)
                units.extend(trs)  # all 4 DMAs go out before any PE work
                for dc in range(DC):

                    def proj_u(dc=dc):
                        pp = ps_y.tile([P, QW], f32, tag="y", name="pp")
                        for c in range(KC):
                            nc.tensor.matmul(
                                pp,
                                wq_sb[:, c, dc * P : (dc + 1) * P],
                                xq_t[:, c, :],
                                start=(c == 0),
                                stop=(c == KC - 1),
                            )
                        nc.vector.tensor_scalar_add(
                            qt_sb[:, dc, tq * QW : (tq + 1) * QW],
                            pp,
                            bq_sb[:, dc : dc + 1],
                        )

                    units.append(proj_u)
                return units

            def po_units(tq, on_act=False):
                """Output-projection work units for `tq` (yt must be final)."""
                units = []
                for ts_ in range(4):
                    tch = tq * 4 + ts_
                    for co in range(2):

                        def u(tch=tch, co=co):
                            po = ps_y.tile([P, QW], f32, tag="y", name="po")
                            for dc in range(DC):
                                nc.tensor.matmul(
                                    po,
                                    yt_sb[:, dc, tch * P : (tch + 1) * P],
                                    wo_sb[:, dc, co * QW : (co + 1) * QW],
                                    start=(dc == 0),
                                    stop=(dc == DC - 1),
                                )
                            o_st = outst.tile([P, QW], f32, tag="o")
                            if on_act and (ts_ + co) % 2 == 0:
                                nc.scalar.copy(o_st, po)
                            else:
                                nc.vector.tensor_copy(o_st, po)
                            # final batch alternates HWDGE/SWDGE so the two
                            # descriptor generators overlap in the tail
                            dma_q = nc.sync
                            dma_q.dma_start(
                                out_d[
                                    tch * P : (tch + 1) * P, co * QW : (co + 1) * QW
                                ],
                                o_st,
                            )

                        units.append(u)
                return units

            # ---------- attention streaming machinery ----------
            y_tiles = {}
            e_tiles = {}
            yq = []  # FIFO of (k, hc, tk) awaiting their AV matmuls
            unit_q = []  # (tag, thunk) PE filler work units
            s1c = [0, 0]  # held S(1,hc) stream cursors during phase A

            def emit_sexp(k, hc, tk):
                sp = ps_s.tile([P, 2 * QW], f32, tag="s", name="sp")
                for hh in range(2):
                    nc.tensor.matmul(
                        sp[:, hh * QW : (hh + 1) * QW],
                        kt_sb[hh * 64 : (hh + 1) * 64, hc, tk * P : (tk + 1) * P],
                        qt_sb[hh * 64 : (hh + 1) * 64, hc, k * QW : (k + 1) * QW],
                        start=True,
                        stop=True,
                        tile_position=(hh * 64, 0),
                    )
                e2 = ework.tile([P, 2 * QW], bf16, tag="e", name="e2")
                nc.scalar.activation(e2, sp, Exp, scale=0.125)
                e_tiles[(k, hc, tk)] = e2

            def emit_y(k, hc, tk):
                """AV partials: exp(S) chunk as stationary, V|1 as moving."""
                if (k, hc) not in y_tiles:
                    y_tiles[(k, hc)] = [
                        ps_acc.tile([P, 4, 65], f32, tag=f"acc{i}", name=f"y_ps{i}")
                        for i in range(2)
                    ]
                y_pair = y_tiles[(k, hc)]
                e2 = e_tiles.pop((k, hc, tk))
                for hh in range(2):
                    h = 2 * hc + hh
                    for cq in range(4):
                        # one accumulation group per PSUM bank (= per hh
                        # tile): start zeroes the whole 2KB zero-region, so
                        # only the very first matmul into the bank starts
                        # and only the very last stops
                        nc.tensor.matmul(
                            y_pair[hh][:, cq, :],
                            e2[:, hh * QW + cq * P : hh * QW + (cq + 1) * P],
                            v_sb[:, tk, h, 0:65],
                            start=(tk == 0 and cq == 0),
                            stop=(tk == NT - 1 and cq == 3),
                        )

            def emit_norm(k, hc, fuse_po=False):
                """Normalize by the ridden-along denominator; build y^T.
                With fuse_po (final pass), each 128-token chunk's output
                projection is emitted the moment its y^T slice lands."""
                y_pair = y_tiles.pop((k, hc))
                den = norm2.tile([P, 2, 4], f32, tag="den")
                for hh in range(2):
                    nc.vector.tensor_copy(den[:, hh, :], y_pair[hh][:, :, 64])
                rec = norm2.tile([P, 2, 4], f32, tag="rec")
                with nc.allow_low_precision(reason="softmax denom reciprocal"):
                    nc.vector.reciprocal(rec, den)
                y2 = norm2.tile([P, 4, P], bf16, tag="y2")
                for hh in range(2):
                    for cq in range(4):
                        nc.vector.tensor_scalar_mul(
                            y2[:, cq, hh * 64 : (hh + 1) * 64],
                            y_pair[hh][:, cq, 0:64],
                            rec[:, hh, cq : cq + 1],
                        )
                tp_y = ps_y.tile([P, 4, P], bf16, tag="y", name="tp_y")
                for cq in range(4):
                    nc.tensor.transpose(tp_y[:, cq, :], y2[:, cq, :], identb)
                nc.vector.tensor_copy(
                    yt_sb[:, hc, k * QW : (k + 1) * QW],
                    tp_y.rearrange("p c t -> p (c t)"),
                )
                if hc == DC - 1 and k < NQ - 1:
                    unit_q.extend(("po", u) for u in po_units(k))

            def attn_step(k, hc, tk, hold=False, lag=LAG):
                """Stream one S/exp step.  hold=True defers the AV work
                entirely (retired later via push_retirement units)."""
                emit_sexp(k, hc, tk)
                if not hold:
                    yq.append((k, hc, tk))
                    if len(yq) > lag:
                        emit_y(*yq.pop(0))

            def push_retirement(k, hc):
                """Queue a held pass's AV matmuls + normalization as filler
                units; they pop during the next pass's S stream."""
                for tk in range(NT):
                    unit_q.append(
                        ("ret", lambda k=k, hc=hc, tk=tk: emit_y(k, hc, tk))
                    )
                unit_q.append(("ret", lambda k=k, hc=hc: emit_norm(k, hc)))

            def pop_units(n):
                for _ in range(n):
                    if unit_q:
                        unit_q.pop(0)[1]()

            # ---- phase A: K^T and V prep with the two tq=0 attention
            # passes woven in (exp starts as soon as the first K half and
            # Q^T(tq0) exist) ----
            # first two x_kv chunk DMAs go out before the x_q block so the
            # PE has transpose work at t=0
            kv_pre = []
            for i in range(2):
                t = xnat.tile([P, C], bf16, tag="xkv_nat", name="kv_nat")
                nc.sync.dma_start(t, xkv_d[i * P : (i + 1) * P, :])
                kv_pre.append(t)
            qunits = q_prep_units(0)
            for u in qunits[:4]:  # xq DMAs up front
                u()
            qrest = qunits[4:]
            # remaining weights after the x loads: their DMA transfers
            # yield the shared DMA engines to the latency-critical x_q path
            nc.gpsimd.dma_start(bq_sb, bq_d.rearrange("(o p) -> p o", p=P))
            nc.gpsimd.dma_start(bk_sb, bk_d.rearrange("(o p) -> p o", p=P))
            nc.gpsimd.dma_start(wo_sb, wo_d.rearrange("(o p) n -> p o n", p=P))
            for tq in range(NQ):
                xkv_t = xtp.tile([P, KC, QW], bf16, tag="xkvT", name="xkv_t")
                for ts_ in range(4):
                    tch = tq * 4 + ts_
                    if tch < len(kv_pre):
                        kv_nat = kv_pre[tch]
                    else:
                        kv_nat = xnat.tile(
                            [P, C], bf16, tag="xkv_nat", name="kv_nat"
                        )
                        nc.sync.dma_start(
                            kv_nat, xkv_d[tch * P : (tch + 1) * P, :]
                        )
                    tp = ps_y.tile([P, KC * P], bf16, tag="y", name="tp8")
                    for c in range(KC):
                        nc.tensor.transpose(
                            tp[:, c * P : (c + 1) * P],
                            kv_nat[:, c * P : (c + 1) * P],
                            identb,
                        )
                    nc.vector.tensor_copy(
                        xkv_t[:, :, ts_ * P : (ts_ + 1) * P],
                        tp.rearrange("p (c t) -> p c t", c=KC),
                    )
                    # finish Q-prep(0) before the first K half completes
                    while qrest and ts_ % 2 == 0:
                        qrest.pop(0)()
                    # queue Q-prep(1) early; it must finish before the held
                    # S(1,*) streams start at tq==2
                    if tq == 0 and ts_ == 0:
                        unit_q.extend(("qp1", u) for u in q_prep_units(1))
                    # K projection first on odd chunks (its DVE bias-add
                    # gates the S matmuls; V's copies can wait)
                    if ts_ in (1, 3):
                        half = ts_ // 2
                        hsl = slice(half * 256, (half + 1) * 256)
                        for dc in range(DC):
                            pp = ps_y.tile([P, 256], f32, tag="y", name="ppk")
                            for c in range(KC):
                                nc.tensor.matmul(
                                    pp,
                                    wk_sb[:, c, dc * P : (dc + 1) * P],
                                    xkv_t[:, c, hsl],
                                    start=(c == 0),
                                    stop=(c == KC - 1),
                                )
                            nc.vector.tensor_scalar_add(
                                kt_sb[:, dc, tq * QW + half * 256 : tq * QW + (half + 1) * 256],
                                pp,
                                bk_sb[:, dc : dc + 1],
                            )
                    # V projection for this token chunk
                    pv = ps_y.tile([P, CL], f32, tag="y", name="pv")
                    for c in range(KC):
                        nc.tensor.matmul(
                            pv,
                            xkv_t[:, c, ts_ * P : (ts_ + 1) * P],
                            wv_sb[:, c, :],
                            start=(c == 0),
                            stop=(c == KC - 1),
                        )
                    nc.vector.tensor_copy(
                        v_sb[:, tch, :, 0:64],
                        pv.rearrange("p (h d) -> p h d", h=HL),
                    )
                    nc.vector.tensor_copy(v_sb[:, tch, :, 64:65], ones4_f32)
                    pop_units(2 if ts_ % 2 == 0 else 1)
                    if tq == 2 and ts_ == 1:
                        # correctness: Q^T(1) writes must be emitted before
                        # the held S(1,*) streams read them
                        while any(t == "qp1" for t, _ in unit_q):
                            pop_units(1)
                    if ts_ in (1, 3):
                        half = ts_ // 2
                        # attention on the two freshly available tk chunks.
                        # Pass (0,0) runs with live (lagged) AV; (0,1) and,
                        # once Q^T(1) exists, (1,0)/(1,1) stream S/exp held
                        # so the ACT exp pipeline is dense through phase A.
                        for tkn in (
                            tq * 4 + half * 2,
                            tq * 4 + half * 2 + 1,
                        ):
                            attn_step(0, 0, tkn)
                        for tkn in (
                            tq * 4 + half * 2,
                            tq * 4 + half * 2 + 1,
                        ):
                            attn_step(0, 1, tkn, hold=True)
                        if tq >= 2:
                            for hcx in range(DC):
                                for _ in range(2):
                                    if s1c[hcx] < NT - 8:
                                        attn_step(1, hcx, s1c[hcx], hold=True)
                                        s1c[hcx] += 1

            # finish the held S(1,*) streams, retire the phase-A passes
            unit_q.extend(("qp2", u) for u in q_prep_units(2))
            for hcx in range(DC):
                for tkx in range(s1c[hcx], NT):
                    attn_step(1, hcx, tkx, hold=True)
                    pop_units(1)
            while yq:
                emit_y(*yq.pop(0))
            emit_norm(0, 0)
            push_retirement(0, 1)
            push_retirement(1, 0)
            push_retirement(1, 1)

            # ---- phase B: remaining S/exp streams; AV+norm of each pass
            # retires as filler during the next pass's stream ----
            for k, hc in [(2, 0), (2, 1), (3, 0), (3, 1)]:
                last = k == NQ - 1 and hc == DC - 1
                if hc == 0:
                    # Q-prep(k) must be fully emitted before S reads Q^T(k)
                    while any(t == f"qp{k}" for t, _ in unit_q):
                        pop_units(1)
                    if k + 1 < NQ:
                        unit_q.extend(
                            (f"qp{k + 1}", u) for u in q_prep_units(k + 1)
                        )
                for tk in range(NT):
                    attn_step(k, hc, tk, hold=True)
                    pop_units(3 if len(unit_q) > 30 else 2)
                if not last:
                    push_retirement(k, hc)
            while unit_q:
                pop_units(1)
            for tk in range(NT):
                emit_y(NQ - 1, DC - 1, tk)
            emit_norm(NQ - 1, DC - 1)
            for u in po_units(NQ - 1, on_act=True):
                u()
            while unit_q:
                pop_units(1)

            ps_y.release()
            ps_acc.release()
            ps_s.release()

    nc.compile()
    return nc


def _get_nc():
    if "nc" not in _CACHE:
        _CACHE["nc"] = _build()
    return _CACHE["nc"]


def _shard_inputs(x_q, x_kv, Wq, bq, Wkv, bkv, Wo):
    import ml_dtypes

    bf = ml_dtypes.bfloat16
    in_maps = []
    for core in range(NCORES):
        b = core // TPG
        g = core % TPG
        cols = slice(g * CL, (g + 1) * CL)
        in_maps.append(
            {
                "xq": np.ascontiguousarray(x_q[b].astype(bf)),
                "xkv": np.ascontiguousarray(x_kv[b].astype(bf)),
                "wq": np.ascontiguousarray(Wq[:, cols].astype(bf)),
                "wk": np.ascontiguousarray(Wkv[:, :C][:, cols].astype(bf)),
                "wv": np.ascontiguousarray(Wkv[:, C:][:, cols].astype(bf)),
                "wo": np.ascontiguousarray(Wo[g * CL : (g + 1) * CL, :].astype(bf)),
                "bq": np.ascontiguousarray(bq[cols]),
                "bk": np.ascontiguousarray(bkv[:C][cols]),
            }
        )
    return in_maps


def kernel(x_q, x_kv, Wq, bq, Wkv, bkv, Wo, bo):
    from concourse.bass_utils import run_bass_kernel_spmd

    x_q = np.asarray(x_q, dtype=np.float32)
    x_kv = np.asarray(x_kv, dtype=np.float32)
    Wq = np.asarray(Wq, dtype=np.float32)
    bq = np.asarray(bq, dtype=np.float32)
    Wkv = np.asarray(Wkv, dtype=np.float32)
    bkv = np.asarray(bkv, dtype=np.float32)
    Wo = np.asarray(Wo, dtype=np.float32)
    bo = np.asarray(bo, dtype=np.float32)

    nc = _get_nc()
    in_maps = _shard_inputs(x_q, x_kv, Wq, bq, Wkv, bkv, Wo)

    res = run_bass_kernel_spmd(nc, in_maps, core_ids=list(range(NCORES)))

    # host-side gather: sum tensor-parallel partials; add exact bias terms
    bias_full = bkv[C:] @ Wo + bo  # v-bias through Wo, plus output bias
    out = np.zeros((B, T, C), dtype=np.float32)
    for core in range(NCORES):
        out[core // TPG] += res.results[core]["out"]
    out += bias_full[None, None, :]
    return out


# revision 5
# speedup vs baseline: 1.0020x; 1.0010x over previous
"""Cross-attention Bass/Tile kernel for Trainium2, sharded over 8 NeuronCores.

Problem (fixed shapes): B=2, T=2048, C=1024, H=16 heads, D=64.
    q = x_q @ Wq + bq;  kv = x_kv @ Wkv + bkv;  k, v = split(kv)
    y = softmax(q k^T / sqrt(D)) v;  out = y @ Wo + bo
Sharding: 8 cores = 2 (batch) x 4 (head groups of 4 heads, 256 channels).

Fully bf16 dataflow (host casts x and weights; rel-err budget 2e-2 >> bf16
noise; PSUM accumulation stays fp32 except the single-shot S logits which
land in bf16 PSUM to halve bank usage).  Attention AV uses exp(S) as the
matmul *stationary* ([tk,128] x [tk,65] -> [tq,65]) so each product streams
65 moving columns instead of 512 (AV: 131k -> 67k PE cycles).  The softmax
denominator rides as a ones-column of V and lands per-partition; the
normalization is a DVE reciprocal + per-partition scalar multiply.
Normalized y transposes back to y^T with bf16 PE transposes.

Schedule: the two tq=0 attention passes are woven *into* phase A (K/V
prep) so the Activation engine's exp stream starts ~10us in; Q-prep for
tq+1 and the deferred output projection weave into the attention passes
as PE filler.  PSUM: 2 x [128,1024 bf16] S slots (2 banks), 4 x
[128,4,65 f32] y accumulators (4), 2 x 2KB weave slots (2).
"""

import numpy as np

B = 2
T = 2048
C = 1024
H = 16
D = 64
NCORES = 8
TPG = 4  # tensor-parallel group size (head groups)
HL = H // TPG  # heads per core = 4
CL = HL * D  # local channels = 256
P = 128

_CACHE = {}


def _build():
    import concourse.tile as tile
    from concourse import bacc, mybir
    from concourse.masks import make_identity

    f32 = mybir.dt.float32
    bf16 = mybir.dt.bfloat16
    Exp = mybir.ActivationFunctionType.Exp

    nc = bacc.Bacc("TRN2", target_bir_lowering=False, debug=False)

    xq_d = nc.dram_tensor("xq", [T, C], bf16, kind="ExternalInput")
    xkv_d = nc.dram_tensor("xkv", [T, C], bf16, kind="ExternalInput")
    wq_d = nc.dram_tensor("wq", [C, CL], bf16, kind="ExternalInput")
    wk_d = nc.dram_tensor("wk", [C, CL], bf16, kind="ExternalInput")
    wv_d = nc.dram_tensor("wv", [C, CL], bf16, kind="ExternalInput")
    wo_d = nc.dram_tensor("wo", [CL, C], bf16, kind="ExternalInput")
    bq_d = nc.dram_tensor("bq", [CL], f32, kind="ExternalInput")
    bk_d = nc.dram_tensor("bk", [CL], f32, kind="ExternalInput")
    out_d = nc.dram_tensor("out", [T, C], f32, kind="ExternalOutput")

    KC = C // P  # 8 contraction chunks for the projections
    NT = T // P  # 16 token chunks of 128
    NQ = 4  # tq chunks of 512
    QW = T // NQ  # 512
    DC = CL // P  # 2 chunks of d_local
    LAG = 5

    with tile.TileContext(nc) as tc:
        with (
            tc.tile_pool(name="const", bufs=1) as const,
            tc.tile_pool(name="persist", bufs=1) as persist,
            tc.tile_pool(name="xnat", bufs=4) as xnat,
            tc.tile_pool(name="xt", bufs=1) as xtp,
            tc.tile_pool(name="ework", bufs=54) as ework,
            tc.tile_pool(name="norm2", bufs=2) as norm2,
            tc.tile_pool(name="outst", bufs=6) as outst,
        ):
            # ---- constants / weights (weights via SWDGE, one DMA per
            # tensor, first-consumer first, so HWDGE is free for x loads
            # and nothing stalls on trickled weight chunks) ----
            ident = const.tile([P, P], f32)
            make_identity(nc, ident)
            identb = const.tile([P, P], bf16)
            nc.vector.tensor_copy(identb, ident)
            ones4_f32 = const.tile([P, HL, 1], f32)
            nc.vector.memset(ones4_f32, 1.0)

            wq_sb = const.tile([P, KC, CL], bf16)
            wk_sb = const.tile([P, KC, CL], bf16)
            wv_sb = const.tile([P, KC, CL], bf16)
            wo_sb = const.tile([P, DC, C], bf16)
            for w_sb, w_d in ((wv_sb, wv_d), (wq_sb, wq_d), (wk_sb, wk_d)):
                nc.gpsimd.dma_start(
                    w_sb, w_d.rearrange("(o p) d -> p o d", p=P)
                )
            bq_sb = const.tile([P, DC], f32)
            bk_sb = const.tile([P, DC], f32)

            # ---- persistent activations ----
            qt_sb = persist.tile([P, DC, T], bf16)  # Q^T  [d, t]
            kt_sb = persist.tile([P, DC, T], bf16)  # K^T  [d, t]
            v_sb = persist.tile([P, NT, HL, 66], bf16)  # V|1 [t, h, d+1]
            yt_sb = persist.tile([P, DC, T], bf16)  # y^T  [d, t] (normalized)

            # ---- kernel-wide PSUM ----
            ps_s = tc.alloc_tile_pool(name="ps_s", bufs=2, space="PSUM")
            ps_acc = tc.alloc_tile_pool(name="ps_acc", bufs=1, space="PSUM")
            ps_y = tc.alloc_tile_pool(name="ps_y", bufs=2, space="PSUM")

            # ---------- emission helpers ----------
            def q_prep_units(tq):
                """Work units (thunks) producing xq^T and Q^T for `tq`."""
                xq_t = xtp.tile([P, KC, QW], bf16, tag="xqT", name="xq_t")
                units = []
                trs = []
                state = {}
                for ts_ in range(4):
                    tch = tq * 4 + ts_

                    def dma_u(ts_=ts_, tch=tch):
                        x_nat = xnat.tile([P, C], bf16, tag="xq_nat", name="x_nat")
                        state[ts_] = x_nat
                        nc.sync.dma_start(x_nat, xq_d[tch * P : (tch + 1) * P, :])

                    units.append(dma_u)
                    for grp in range(2):

                        def tr_u(ts_=ts_, grp=grp):
                            x_nat = state[ts_]
                            tp = ps_y.tile([P, 4 * P], bf16, tag="y", name="tp")
                            for cc in range(4):
                                c = grp * 4 + cc
                                nc.tensor.transpose(
                                    tp[:, cc * P : (cc + 1) * P],
                                    x_nat[:, c * P : (c + 1) * P],
                                    identb,
                                )
                            nc.vector.tensor_copy(
                                xq_t[
                                    :, grp * 4 : (grp + 1) * 4, ts_ * P : (ts_ + 1) * P
                                ],
                                tp.rearrange("p (c t) -> p c t", c=4),
                            )

                        trs.append(tr_u)
                units.extend(trs)  # all 4 DMAs go out before any PE work
                for dc in range(DC):

                    def proj_u(dc=dc):
                        pp = ps_y.tile([P, QW], f32, tag="y", name="pp")
                        for c in range(KC):
                            nc.tensor.matmul(
                                pp,
                                wq_sb[:, c, dc * P : (dc + 1) * P],
                                xq_t[:, c, :],
                                start=(c == 0),
                                stop=(c == KC - 1),
                            )
                        nc.vector.tensor_scalar_add(
                            qt_sb[:, dc, tq * QW : (tq + 1) * QW],
                            pp,
                            bq_sb[:, dc : dc + 1],
                        )

                    units.append(proj_u)
                return units

            def po_units(tq, on_act=False):
                """Output-projection work units for `tq` (yt must be final)."""
                units = []
                for ts_ in range(4):
                    tch = tq * 4 + ts_
                    for co in range(2):

                        def u(tch=tch, co=co):
                            po = ps_y.tile([P, QW], f32, tag="y", name="po")
                            for dc in range(DC):
                                nc.tensor.matmul(
                                    po,
                                    yt_sb[:, dc, tch * P : (tch + 1) * P],
                                    wo_sb[:, dc, co * QW : (co + 1) * QW],
                                    start=(dc == 0),
                                    stop=(dc == DC - 1),
                                )
                            o_st = outst.tile([P, QW], f32, tag="o")
                            if on_act and (ts_ + co) % 2 == 0:
                                nc.scalar.copy(o_st, po)
                            else:
                                nc.vector.tensor_copy(o_st, po)
                            # final batch alternates HWDGE/SWDGE so the two
                            # descriptor generators overlap in the tail
                            dma_q = nc.sync
                            dma_q.dma_start(
                                out_d[
                                    tch * P : (tch + 1) * P, co * QW : (co + 1) * QW
                                ],
                                o_st,
                            )

                        units.append(u)
                return units

            # ---------- attention streaming machinery ----------
            y_tiles = {}
            e_tiles = {}
            yq = []  # FIFO of (k, hc, tk) awaiting their AV matmuls
            unit_q = []  # (tag, thunk) PE filler work units
            s1c = [0, 0]  # held S(1,hc) stream cursors during phase A

            def emit_sexp(k, hc, tk):
                sp = ps_s.tile([P, 2 * QW], f32, tag="s", name="sp")
                for hh in range(2):
                    nc.tensor.matmul(
                        sp[:, hh * QW : (hh + 1) * QW],
                        kt_sb[hh * 64 : (hh + 1) * 64, hc, tk * P : (tk + 1) * P],
                        qt_sb[hh * 64 : (hh + 1) * 64, hc, k * QW : (k + 1) * QW],
                        start=True,
                        stop=True,
                        tile_position=(hh * 64, 0),
                    )
                e2 = ework.tile([P, 2 * QW], bf16, tag="e", name="e2")
                nc.scalar.activation(e2, sp, Exp, scale=0.125)
                e_tiles[(k, hc, tk)] = e2

            def emit_y(k, hc, tk):
                """AV partials: exp(S) chunk as stationary, V|1 as moving."""
                if (k, hc) not in y_tiles:
                    y_tiles[(k, hc)] = [
                        ps_acc.tile([P, 4, 65], f32, tag=f"acc{i}", name=f"y_ps{i}")
                        for i in range(2)
                    ]
                y_pair = y_tiles[(k, hc)]
                e2 = e_tiles.pop((k, hc, tk))
                for hh in range(2):
                    h = 2 * hc + hh
                    for cq in range(4):
                        # one accumulation group per PSUM bank (= per hh
                        # tile): start zeroes the whole 2KB zero-region, so
                        # only the very first matmul into the bank starts
                        # and only the very last stops
                        nc.tensor.matmul(
                            y_pair[hh][:, cq, :],
                            e2[:, hh * QW + cq * P : hh * QW + (cq + 1) * P],
                            v_sb[:, tk, h, 0:65],
                            start=(tk == 0 and cq == 0),
                            stop=(tk == NT - 1 and cq == 3),
                        )

            def emit_norm(k, hc, fuse_po=False):
                """Normalize by the ridden-along denominator; build y^T.
                With fuse_po (final pass), each 128-token chunk's output
                projection is emitted the moment its y^T slice lands."""
                y_pair = y_tiles.pop((k, hc))
                den = norm2.tile([P, 2, 4], f32, tag="den")
                for hh in range(2):
                    nc.vector.tensor_copy(den[:, hh, :], y_pair[hh][:, :, 64])
                rec = norm2.tile([P, 2, 4], f32, tag="rec")
                with nc.allow_low_precision(reason="softmax denom reciprocal"):
                    nc.vector.reciprocal(rec, den)
                y2 = norm2.tile([P, 4, P], bf16, tag="y2")
                for hh in range(2):
                    for cq in range(4):
                        nc.vector.tensor_scalar_mul(
                            y2[:, cq, hh * 64 : (hh + 1) * 64],
                            y_pair[hh][:, cq, 0:64],
                            rec[:, hh, cq : cq + 1],
                        )
                tp_y = ps_y.tile([P, 4, P], bf16, tag="y", name="tp_y")
                for cq in range(4):
                    nc.tensor.transpose(tp_y[:, cq, :], y2[:, cq, :], identb)
                nc.vector.tensor_copy(
                    yt_sb[:, hc, k * QW : (k + 1) * QW],
                    tp_y.rearrange("p c t -> p (c t)"),
                )
                if hc == DC - 1 and k < NQ - 1:
                    unit_q.extend(("po", u) for u in po_units(k))

            def attn_step(k, hc, tk, hold=False, lag=LAG):
                """Stream one S/exp step.  hold=True defers the AV work
                entirely (retired later via push_retirement units)."""
                emit_sexp(k, hc, tk)
                if not hold:
                    yq.append((k, hc, tk))
                    if len(yq) > lag:
                        emit_y(*yq.pop(0))

            def push_retirement(k, hc):
                """Queue a held pass's AV matmuls + normalization as filler
                units; they pop during the next pass's S stream."""
                for tk in range(NT):
                    unit_q.append(
                        ("ret", lambda k=k, hc=hc, tk=tk: emit_y(k, hc, tk))
                    )
                unit_q.append(("ret", lambda k=k, hc=hc: emit_norm(k, hc)))

            def pop_units(n):
                for _ in range(n):
                    if unit_q:
                        unit_q.pop(0)[1]()

            # ---- phase A: K^T and V prep with the two tq=0 attention
            # passes woven in (exp starts as soon as the first K half and
            # Q^T(tq0) exist) ----
            # first two x_kv chunk DMAs go out before the x_q block so the
            # PE has transpose work at t=0
            kv_pre = []
            for i in range(2):
                t = xnat.tile([P, C], bf16, tag="xkv_nat", name="kv_nat")
                nc.scalar.dma_start(t, xkv_d[i * P : (i + 1) * P, :])
                kv_pre.append(t)
            qunits = q_prep_units(0)
            for u in qunits[:4]:  # xq DMAs up front
                u()
            qrest = qunits[4:]
            # remaining weights after the x loads: their DMA transfers
            # yield the shared DMA engines to the latency-critical x_q path
            nc.gpsimd.dma_start(bq_sb, bq_d.rearrange("(o p) -> p o", p=P))
            nc.gpsimd.dma_start(bk_sb, bk_d.rearrange("(o p) -> p o", p=P))
            nc.gpsimd.dma_start(wo_sb, wo_d.rearrange("(o p) n -> p o n", p=P))
            for tq in range(NQ):
                xkv_t = xtp.tile([P, KC, QW], bf16, tag="xkvT", name="xkv_t")
                for ts_ in range(4):
                    tch = tq * 4 + ts_
                    if tch < len(kv_pre):
                        kv_nat = kv_pre[tch]
                    else:
                        kv_nat = xnat.tile(
                            [P, C], bf16, tag="xkv_nat", name="kv_nat"
                        )
                        nc.sync.dma_start(
                            kv_nat, xkv_d[tch * P : (tch + 1) * P, :]
                        )
                    tp = ps_y.tile([P, KC * P], bf16, tag="y", name="tp8")
                    for c in range(KC):
                        nc.tensor.transpose(
                            tp[:, c * P : (c + 1) * P],
                            kv_nat[:, c * P : (c + 1) * P],
                            identb,
                        )
                    nc.vector.tensor_copy(
                        xkv_t[:, :, ts_ * P : (ts_ + 1) * P],
                        tp.rearrange("p (c t) -> p c t", c=KC),
                    )
                    # finish Q-prep(0) before the first K half completes
                    while qrest and ts_ % 2 == 0:
                        qrest.pop(0)()
                    # queue Q-prep(1) early; it must finish before the held
                    # S(1,*) streams start at tq==2
                    if tq == 0 and ts_ == 0:
                        unit_q.extend(("qp1", u) for u in q_prep_units(1))
                    # K projection first on odd chunks (its DVE bias-add
                    # gates the S matmuls; V's copies can wait)
                    if ts_ in (1, 3):
                        half = ts_ // 2
                        hsl = slice(half * 256, (half + 1) * 256)
                        for dc in range(DC):
                            pp = ps_y.tile([P, 256], f32, tag="y", name="ppk")
                            for c in range(KC):
                                nc.tensor.matmul(
                                    pp,
                                    wk_sb[:, c, dc * P : (dc + 1) * P],
                                    xkv_t[:, c, hsl],
                                    start=(c == 0),
                                    stop=(c == KC - 1),
                                )
                            nc.vector.tensor_scalar_add(
                                kt_sb[:, dc, tq * QW + half * 256 : tq * QW + (half + 1) * 256],
                                pp,
                                bk_sb[:, dc : dc + 1],
                            )
                    # V projection for this token chunk
                    pv = ps_y.tile([P, CL], f32, tag="y", name="pv")
                    for c in range(KC):
                        nc.tensor.matmul(
                            pv,
                            xkv_t[:, c, ts_ * P : (ts_ + 1) * P],
                            wv_sb[:, c, :],
                            start=(c == 0),
                            stop=(c == KC - 1),
                        )
                    nc.vector.tensor_copy(
                        v_sb[:, tch, :, 0:64],
                        pv.rearrange("p (h d) -> p h d", h=HL),
                    )
                    nc.vector.tensor_copy(v_sb[:, tch, :, 64:65], ones4_f32)
                    pop_units(2 if ts_ % 2 == 0 else 1)
                    if tq == 2 and ts_ == 1:
                        # correctness: Q^T(1) writes must be emitted before
                        # the held S(1,*) streams read them
                        while any(t == "qp1" for t, _ in unit_q):
                            pop_units(1)
                    if ts_ in (1, 3):
                        half = ts_ // 2
                        # attention on the two freshly available tk chunks.
                        # Pass (0,0) runs with live (lagged) AV; (0,1) and,
                        # once Q^T(1) exists, (1,0)/(1,1) stream S/exp held
                        # so the ACT exp pipeline is dense through phase A.
                        for tkn in (
                            tq * 4 + half * 2,
                            tq * 4 + half * 2 + 1,
                        ):
                            attn_step(0, 0, tkn)
                        for tkn in (
                            tq * 4 + half * 2,
                            tq * 4 + half * 2 + 1,
                        ):
                            attn_step(0, 1, tkn, hold=True)
                        if tq >= 2:
                            for hcx in range(DC):
                                for _ in range(2):
                                    if s1c[hcx] < NT - 8:
                                        attn_step(1, hcx, s1c[hcx], hold=True)
                                        s1c[hcx] += 1

            # finish the held S(1,*) streams, retire the phase-A passes
            unit_q.extend(("qp2", u) for u in q_prep_units(2))
            for hcx in range(DC):
                for tkx in range(s1c[hcx], NT):
                    attn_step(1, hcx, tkx, hold=True)
                    pop_units(1)
            while yq:
                emit_y(*yq.pop(0))
            emit_norm(0, 0)
            push_retirement(0, 1)
            push_retirement(1, 0)
            push_retirement(1, 1)

            # ---- phase B: remaining S/exp streams; AV+norm of each pass
            # retires as filler during the next pass's stream ----
            for k, hc in [(2, 0), (2, 1), (3, 0), (3, 1)]:
                last = k == NQ - 1 and hc == DC - 1
                if hc == 0:
                    # Q-prep(k) must be fully emitted before S reads Q^T(k)
                    while any(t == f"qp{k}" for t, _ in unit_q):
                        pop_units(1)
                    if k + 1 < NQ:
                        unit_q.extend(
                            (f"qp{k + 1}", u) for u in q_prep_units(k + 1)
                        )
                for tk in range(NT):
                    attn_step(k, hc, tk, hold=True)
                    pop_units(3 if len(unit_q) > 30 else 2)
                if not last:
                    push_retirement(k, hc)
            while unit_q:
                pop_units(1)
            for tk in range(NT):
                emit_y(NQ - 1, DC - 1, tk)
            emit_norm(NQ - 1, DC - 1)
            for u in po_units(NQ - 1, on_act=True):
                u()
            while unit_q:
                pop_units(1)

            ps_y.release()
            ps_acc.release()
            ps_s.release()

    nc.compile()
    return nc


def _get_nc():
    if "nc" not in _CACHE:
        _CACHE["nc"] = _build()
    return _CACHE["nc"]


def _shard_inputs(x_q, x_kv, Wq, bq, Wkv, bkv, Wo):
    import ml_dtypes

    bf = ml_dtypes.bfloat16
    in_maps = []
    for core in range(NCORES):
        b = core // TPG
        g = core % TPG
        cols = slice(g * CL, (g + 1) * CL)
        in_maps.append(
            {
                "xq": np.ascontiguousarray(x_q[b].astype(bf)),
                "xkv": np.ascontiguousarray(x_kv[b].astype(bf)),
                "wq": np.ascontiguousarray(Wq[:, cols].astype(bf)),
                "wk": np.ascontiguousarray(Wkv[:, :C][:, cols].astype(bf)),
                "wv": np.ascontiguousarray(Wkv[:, C:][:, cols].astype(bf)),
                "wo": np.ascontiguousarray(Wo[g * CL : (g + 1) * CL, :].astype(bf)),
                "bq": np.ascontiguousarray(bq[cols]),
                "bk": np.ascontiguousarray(bkv[:C][cols]),
            }
        )
    return in_maps


def kernel(x_q, x_kv, Wq, bq, Wkv, bkv, Wo, bo):
    from concourse.bass_utils import run_bass_kernel_spmd

    x_q = np.asarray(x_q, dtype=np.float32)
    x_kv = np.asarray(x_kv, dtype=np.float32)
    Wq = np.asarray(Wq, dtype=np.float32)
    bq = np.asarray(bq, dtype=np.float32)
    Wkv = np.asarray(Wkv, dtype=np.float32)
    bkv = np.asarray(bkv, dtype=np.float32)
    Wo = np.asarray(Wo, dtype=np.float32)
    bo = np.asarray(bo, dtype=np.float32)

    nc = _get_nc()
    in_maps = _shard_inputs(x_q, x_kv, Wq, bq, Wkv, bkv, Wo)

    res = run_bass_kernel_spmd(nc, in_maps, core_ids=list(range(NCORES)))

    # host-side gather: sum tensor-parallel partials; add exact bias terms
    bias_full = bkv[C:] @ Wo + bo  # v-bias through Wo, plus output bias
    out = np.zeros((B, T, C), dtype=np.float32)
    for core in range(NCORES):
        out[core // TPG] += res.results[core]["out"]
    out += bias_full[None, None, :]
    return out


# revision 7
# speedup vs baseline: 1.0238x; 1.0218x over previous
"""Cross-attention Bass/Tile kernel for Trainium2, sharded over 8 NeuronCores.

Problem (fixed shapes): B=2, T=2048, C=1024, H=16 heads, D=64.
    q = x_q @ Wq + bq;  kv = x_kv @ Wkv + bkv;  k, v = split(kv)
    y = softmax(q k^T / sqrt(D)) v;  out = y @ Wo + bo
Sharding: 8 cores = 2 (batch) x 4 (head groups of 4 heads, 256 channels).

Fully bf16 dataflow (host casts x and weights; rel-err budget 2e-2 >> bf16
noise; PSUM accumulation stays fp32 except the single-shot S logits which
land in bf16 PSUM to halve bank usage).  Attention AV uses exp(S) as the
matmul *stationary* ([tk,128] x [tk,65] -> [tq,65]) so each product streams
65 moving columns instead of 512 (AV: 131k -> 67k PE cycles).  The softmax
denominator rides as a ones-column of V and lands per-partition; the
normalization is a DVE reciprocal + per-partition scalar multiply.
Normalized y transposes back to y^T with bf16 PE transposes.

Schedule: the two tq=0 attention passes are woven *into* phase A (K/V
prep) so the Activation engine's exp stream starts ~10us in; Q-prep for
tq+1 and the deferred output projection weave into the attention passes
as PE filler.  PSUM: 2 x [128,1024 bf16] S slots (2 banks), 4 x
[128,4,65 f32] y accumulators (4), 2 x 2KB weave slots (2).
"""

import numpy as np

B = 2
T = 2048
C = 1024
H = 16
D = 64
NCORES = 8
TPG = 4  # tensor-parallel group size (head groups)
HL = H // TPG  # heads per core = 4
CL = HL * D  # local channels = 256
P = 128

_CACHE = {}


def _build():
    import concourse.tile as tile
    from concourse import bacc, mybir
    from concourse.masks import make_identity

    f32 = mybir.dt.float32
    bf16 = mybir.dt.bfloat16
    Exp = mybir.ActivationFunctionType.Exp

    nc = bacc.Bacc("TRN2", target_bir_lowering=False, debug=False)

    xq_d = nc.dram_tensor("xq", [T, C], bf16, kind="ExternalInput")
    xkv_d = nc.dram_tensor("xkv", [T, C], bf16, kind="ExternalInput")
    wq_d = nc.dram_tensor("wq", [C, CL], bf16, kind="ExternalInput")
    wk_d = nc.dram_tensor("wk", [C, CL], bf16, kind="ExternalInput")
    wv_d = nc.dram_tensor("wv", [C, CL], bf16, kind="ExternalInput")
    wo_d = nc.dram_tensor("wo", [CL, C], bf16, kind="ExternalInput")
    bq_d = nc.dram_tensor("bq", [CL], f32, kind="ExternalInput")
    bk_d = nc.dram_tensor("bk", [CL], f32, kind="ExternalInput")
    out_d = nc.dram_tensor("out", [T, C], f32, kind="ExternalOutput")

    KC = C // P  # 8 contraction chunks for the projections
    NT = T // P  # 16 token chunks of 128
    NQ = 4  # tq chunks of 512
    QW = T // NQ  # 512
    DC = CL // P  # 2 chunks of d_local
    LAG = 5

    with tile.TileContext(nc) as tc:
        with (
            tc.tile_pool(name="const", bufs=1) as const,
            tc.tile_pool(name="persist", bufs=1) as persist,
            tc.tile_pool(name="xnat", bufs=4) as xnat,
            tc.tile_pool(name="xt", bufs=1) as xtp,
            tc.tile_pool(name="ework", bufs=54) as ework,
            tc.tile_pool(name="norm2", bufs=2) as norm2,
            tc.tile_pool(name="outst", bufs=6) as outst,
        ):
            # ---- constants / weights (weights via SWDGE, one DMA per
            # tensor, first-consumer first, so HWDGE is free for x loads
            # and nothing stalls on trickled weight chunks) ----
            ident = const.tile([P, P], f32)
            make_identity(nc, ident)
            identb = const.tile([P, P], bf16)
            nc.vector.tensor_copy(identb, ident)
            ones4_f32 = const.tile([P, HL, 1], f32)
            nc.vector.memset(ones4_f32, 1.0)

            wq_sb = const.tile([P, KC, CL], bf16)
            wk_sb = const.tile([P, KC, CL], bf16)
            wv_sb = const.tile([P, KC, CL], bf16)
            wo_sb = const.tile([P, DC, C], bf16)
            for w_sb, w_d in ((wv_sb, wv_d), (wq_sb, wq_d), (wk_sb, wk_d)):
                nc.gpsimd.dma_start(
                    w_sb, w_d.rearrange("(o p) d -> p o d", p=P)
                )
            bq_sb = const.tile([P, DC], f32)
            bk_sb = const.tile([P, DC], f32)

            # ---- persistent activations ----
            qt_sb = persist.tile([P, DC, T], bf16)  # Q^T  [d, t]
            kt_sb = persist.tile([P, DC, T], bf16)  # K^T  [d, t]
            v_sb = persist.tile([P, NT, HL, 66], bf16)  # V|1 [t, h, d+1]
            yt_sb = persist.tile([P, DC, T], bf16)  # y^T  [d, t] (normalized)

            # ---- kernel-wide PSUM ----
            ps_s = tc.alloc_tile_pool(name="ps_s", bufs=2, space="PSUM")
            ps_acc = tc.alloc_tile_pool(name="ps_acc", bufs=1, space="PSUM")
            ps_y = tc.alloc_tile_pool(name="ps_y", bufs=2, space="PSUM")

            # ---------- emission helpers ----------
            def q_prep_units(tq):
                """Work units (thunks) producing xq^T and Q^T for `tq`."""
                xq_t = xtp.tile([P, KC, QW], bf16, tag="xqT", name="xq_t")
                units = []
                trs = []
                state = {}
                for ts_ in range(4):
                    tch = tq * 4 + ts_

                    def dma_u(ts_=ts_, tch=tch):
                        x_nat = xnat.tile([P, C], bf16, tag="xq_nat", name="x_nat")
                        state[ts_] = x_nat
                        nc.sync.dma_start(x_nat, xq_d[tch * P : (tch + 1) * P, :])

                    units.append(dma_u)
                    for grp in range(2):

                        def tr_u(ts_=ts_, grp=grp):
                            x_nat = state[ts_]
                            tp = ps_y.tile([P, 4 * P], bf16, tag="y", name="tp")
                            for cc in range(4):
                                c = grp * 4 + cc
                                nc.tensor.transpose(
                                    tp[:, cc * P : (cc + 1) * P],
                                    x_nat[:, c * P : (c + 1) * P],
                                    identb,
                                )
                            nc.vector.tensor_copy(
                                xq_t[
                                    :, grp * 4 : (grp + 1) * 4, ts_ * P : (ts_ + 1) * P
                                ],
                                tp.rearrange("p (c t) -> p c t", c=4),
                            )

                        trs.append(tr_u)
                units.extend(trs)  # all 4 DMAs go out before any PE work
                for dc in range(DC):

                    def proj_u(dc=dc):
                        pp = ps_y.tile([P, QW], f32, tag="y", name="pp")
                        for c in range(KC):
                            nc.tensor.matmul(
                                pp,
                                wq_sb[:, c, dc * P : (dc + 1) * P],
                                xq_t[:, c, :],
                                start=(c == 0),
                                stop=(c == KC - 1),
                            )
                        nc.vector.tensor_scalar_add(
                            qt_sb[:, dc, tq * QW : (tq + 1) * QW],
                            pp,
                            bq_sb[:, dc : dc + 1],
                        )

                    units.append(proj_u)
                return units

            def po_units(tq, on_act=False):
                """Output-projection work units for `tq` (yt must be final)."""
                units = []
                for ts_ in range(4):
                    tch = tq * 4 + ts_
                    for co in range(2):

                        def u(tch=tch, co=co):
                            po = ps_y.tile([P, QW], f32, tag="y", name="po")
                            for dc in range(DC):
                                nc.tensor.matmul(
                                    po,
                                    yt_sb[:, dc, tch * P : (tch + 1) * P],
                                    wo_sb[:, dc, co * QW : (co + 1) * QW],
                                    start=(dc == 0),
                                    stop=(dc == DC - 1),
                                )
                            o_st = outst.tile([P, QW], f32, tag="o")
                            if on_act and (ts_ + co) % 2 == 0:
                                nc.scalar.copy(o_st, po)
                            else:
                                nc.vector.tensor_copy(o_st, po)
                            # final batch alternates HWDGE/SWDGE so the two
                            # descriptor generators overlap in the tail
                            dma_q = nc.sync
                            dma_q.dma_start(
                                out_d[
                                    tch * P : (tch + 1) * P, co * QW : (co + 1) * QW
                                ],
                                o_st,
                            )

                        units.append(u)
                return units

            # ---------- attention streaming machinery ----------
            y_tiles = {}
            e_tiles = {}
            yq = []  # FIFO of (k, hc, tk) awaiting their AV matmuls
            unit_q = []  # (tag, thunk) PE filler work units
            s1c = [0, 0]  # held S(1,hc) stream cursors during phase A

            def emit_sexp(k, hc, tk):
                sp = ps_s.tile([P, 2 * QW], f32, tag="s", name="sp")
                for hh in range(2):
                    nc.tensor.matmul(
                        sp[:, hh * QW : (hh + 1) * QW],
                        kt_sb[hh * 64 : (hh + 1) * 64, hc, tk * P : (tk + 1) * P],
                        qt_sb[hh * 64 : (hh + 1) * 64, hc, k * QW : (k + 1) * QW],
                        start=True,
                        stop=True,
                        tile_position=(hh * 64, 0),
                    )
                e2 = ework.tile([P, 2 * QW], bf16, tag="e", name="e2")
                nc.scalar.activation(e2, sp, Exp, scale=0.125)
                e_tiles[(k, hc, tk)] = e2

            def emit_y(k, hc, tk):
                """AV partials: exp(S) chunk as stationary, V|1 as moving."""
                if (k, hc) not in y_tiles:
                    y_tiles[(k, hc)] = [
                        ps_acc.tile([P, 4, 65], f32, tag=f"acc{i}", name=f"y_ps{i}")
                        for i in range(2)
                    ]
                y_pair = y_tiles[(k, hc)]
                e2 = e_tiles.pop((k, hc, tk))
                for hh in range(2):
                    h = 2 * hc + hh
                    for cq in range(4):
                        # one accumulation group per PSUM bank (= per hh
                        # tile): start zeroes the whole 2KB zero-region, so
                        # only the very first matmul into the bank starts
                        # and only the very last stops
                        nc.tensor.matmul(
                            y_pair[hh][:, cq, :],
                            e2[:, hh * QW + cq * P : hh * QW + (cq + 1) * P],
                            v_sb[:, tk, h, 0:65],
                            start=(tk == 0 and cq == 0),
                            stop=(tk == NT - 1 and cq == 3),
                        )

            def emit_norm(k, hc, fuse_po=False):
                """Normalize by the ridden-along denominator; build y^T.
                With fuse_po (final pass), each 128-token chunk's output
                projection is emitted the moment its y^T slice lands."""
                y_pair = y_tiles.pop((k, hc))
                den = norm2.tile([P, 2, 4], f32, tag="den")
                for hh in range(2):
                    nc.vector.tensor_copy(den[:, hh, :], y_pair[hh][:, :, 64])
                rec = norm2.tile([P, 2, 4], f32, tag="rec")
                with nc.allow_low_precision(reason="softmax denom reciprocal"):
                    nc.vector.reciprocal(rec, den)
                y2 = norm2.tile([P, 4, P], bf16, tag="y2")
                for hh in range(2):
                    for cq in range(4):
                        nc.vector.tensor_scalar_mul(
                            y2[:, cq, hh * 64 : (hh + 1) * 64],
                            y_pair[hh][:, cq, 0:64],
                            rec[:, hh, cq : cq + 1],
                        )
                tp_y = ps_y.tile([P, 4, P], bf16, tag="y", name="tp_y")
                for cq in range(4):
                    nc.tensor.transpose(tp_y[:, cq, :], y2[:, cq, :], identb)
                nc.vector.tensor_copy(
                    yt_sb[:, hc, k * QW : (k + 1) * QW],
                    tp_y.rearrange("p c t -> p (c t)"),
                )
                if hc == DC - 1 and k < NQ - 1:
                    unit_q.extend(("po", u) for u in po_units(k))

            def attn_step(k, hc, tk, hold=False, lag=LAG):
                """Stream one S/exp step.  hold=True defers the AV work
                entirely (retired later via push_retirement units)."""
                emit_sexp(k, hc, tk)
                if not hold:
                    yq.append((k, hc, tk))
                    if len(yq) > lag:
                        emit_y(*yq.pop(0))

            def push_retirement(k, hc):
                """Queue a held pass's AV matmuls + normalization as filler
                units; they pop during the next pass's S stream."""
                for tk in range(NT):
                    unit_q.append(
                        ("ret", lambda k=k, hc=hc, tk=tk: emit_y(k, hc, tk))
                    )
                unit_q.append(("ret", lambda k=k, hc=hc: emit_norm(k, hc)))

            def pop_units(n):
                for _ in range(n):
                    if unit_q:
                        unit_q.pop(0)[1]()

            # ---- phase A: K^T and V prep with the two tq=0 attention
            # passes woven in (exp starts as soon as the first K half and
            # Q^T(tq0) exist) ----
            # first two x_kv chunk DMAs go out before the x_q block so the
            # PE has transpose work at t=0
            kv_pre = []
            for i in range(2):
                t = xnat.tile([P, C], bf16, tag="xkv_nat", name="kv_nat")
                nc.scalar.dma_start(t, xkv_d[i * P : (i + 1) * P, :])
                kv_pre.append(t)
            qunits = q_prep_units(0)
            for u in qunits[:4]:  # xq DMAs up front
                u()
            qrest = qunits[4:]
            # remaining weights after the x loads: their DMA transfers
            # yield the shared DMA engines to the latency-critical x_q path
            nc.gpsimd.dma_start(bq_sb, bq_d.rearrange("(o p) -> p o", p=P))
            nc.gpsimd.dma_start(bk_sb, bk_d.rearrange("(o p) -> p o", p=P))
            nc.gpsimd.dma_start(wo_sb, wo_d.rearrange("(o p) n -> p o n", p=P))
            for tq in range(NQ):
                xkv_t = xtp.tile([P, KC, QW], bf16, tag="xkvT", name="xkv_t")
                for half in range(2):
                    # transposes + xkv^T copies for the half's two chunks:
                    # nothing else enters the DVE queue ahead of the
                    # latency-critical xkv^T -> K-proj -> kt-bias chain
                    for s2 in range(2):
                        ts_ = half * 2 + s2
                        tch = tq * 4 + ts_
                        if tch < len(kv_pre):
                            kv_nat = kv_pre[tch]
                        else:
                            kv_nat = xnat.tile(
                                [P, C], bf16, tag="xkv_nat", name="kv_nat"
                            )
                            nc.sync.dma_start(
                                kv_nat, xkv_d[tch * P : (tch + 1) * P, :]
                            )
                        tp = ps_y.tile([P, KC * P], bf16, tag="y", name="tp8")
                        for c in range(KC):
                            nc.tensor.transpose(
                                tp[:, c * P : (c + 1) * P],
                                kv_nat[:, c * P : (c + 1) * P],
                                identb,
                            )
                        nc.vector.tensor_copy(
                            xkv_t[:, :, ts_ * P : (ts_ + 1) * P],
                            tp.rearrange("p (c t) -> p c t", c=KC),
                        )
                    # finish Q-prep(0) before the first K half completes
                    while qrest:
                        qrest.pop(0)()
                    # queue Q-prep(1) early; it must finish before the held
                    # S(1,*) streams start at tq==2
                    if tq == 0 and half == 0:
                        unit_q.extend(("qp1", u) for u in q_prep_units(1))
                    hsl = slice(half * 256, (half + 1) * 256)
                    for dc in range(DC):
                        pp = ps_y.tile([P, 256], f32, tag="y", name="ppk")
                        for c in range(KC):
                            nc.tensor.matmul(
                                pp,
                                wk_sb[:, c, dc * P : (dc + 1) * P],
                                xkv_t[:, c, hsl],
                                start=(c == 0),
                                stop=(c == KC - 1),
                            )
                        nc.vector.tensor_scalar_add(
                            kt_sb[:, dc, tq * QW + half * 256 : tq * QW + (half + 1) * 256],
                            pp,
                            bk_sb[:, dc : dc + 1],
                        )
                    if tq == 2 and half == 0:
                        # correctness: Q^T(1) writes must be emitted before
                        # the held S(1,*) streams read them
                        while any(t == "qp1" for t, _ in unit_q):
                            pop_units(1)
                    # attention on the two freshly available tk chunks.
                    # Pass (0,0) runs with live (lagged) AV; (0,1) and,
                    # once Q^T(1) exists, (1,0)/(1,1) stream S/exp held
                    # so the ACT exp pipeline is dense through phase A.
                    for tkn in (
                        tq * 4 + half * 2,
                        tq * 4 + half * 2 + 1,
                    ):
                        attn_step(0, 0, tkn)
                    for tkn in (
                        tq * 4 + half * 2,
                        tq * 4 + half * 2 + 1,
                    ):
                        attn_step(0, 1, tkn, hold=True)
                    if tq >= 2:
                        for hcx in range(DC):
                            for _ in range(2):
                                if s1c[hcx] < NT - 8:
                                    attn_step(1, hcx, s1c[hcx], hold=True)
                                    s1c[hcx] += 1
                    # deferred V projections + copies for the half's chunks
                    for s2 in range(2):
                        ts_ = half * 2 + s2
                        tch = tq * 4 + ts_
                        pv = ps_y.tile([P, CL], f32, tag="y", name="pv")
                        for c in range(KC):
                            nc.tensor.matmul(
                                pv,
                                xkv_t[:, c, ts_ * P : (ts_ + 1) * P],
                                wv_sb[:, c, :],
                                start=(c == 0),
                                stop=(c == KC - 1),
                            )
                        nc.vector.tensor_copy(
                            v_sb[:, tch, :, 0:64],
                            pv.rearrange("p (h d) -> p h d", h=HL),
                        )
                        nc.vector.tensor_copy(v_sb[:, tch, :, 64:65], ones4_f32)
                    pop_units(1)

            # finish the held S(1,*) streams, retire the phase-A passes
            unit_q.extend(("qp2", u) for u in q_prep_units(2))
            for hcx in range(DC):
                for tkx in range(s1c[hcx], NT):
                    attn_step(1, hcx, tkx, hold=True)
                    pop_units(1)
            while yq:
                emit_y(*yq.pop(0))
            emit_norm(0, 0)
            push_retirement(0, 1)
            push_retirement(1, 0)
            push_retirement(1, 1)

            # ---- phase B: remaining S/exp streams; AV+norm of each pass
            # retires as filler during the next pass's stream ----
            for k, hc in [(2, 0), (2, 1), (3, 0), (3, 1)]:
                last = k == NQ - 1 and hc == DC - 1
                if hc == 0:
                    # Q-prep(k) must be fully emitted before S reads Q^T(k)
                    while any(t == f"qp{k}" for t, _ in unit_q):
                        pop_units(1)
                    if k + 1 < NQ:
                        unit_q.extend(
                            (f"qp{k + 1}", u) for u in q_prep_units(k + 1)
                        )
                for tk in range(NT):
                    attn_step(k, hc, tk, hold=True)
                    pop_units(3 if len(unit_q) > 30 else 2)
                if not last:
                    push_retirement(k, hc)
            while unit_q:
                pop_units(1)
            for tk in range(NT):
                emit_y(NQ - 1, DC - 1, tk)
            emit_norm(NQ - 1, DC - 1)
            for u in po_units(NQ - 1, on_act=True):
                u()
            while unit_q:
                pop_units(1)

            ps_y.release()
            ps_acc.release()
            ps_s.release()

    nc.compile()
    return nc


def _get_nc():
    if "nc" not in _CACHE:
        _CACHE["nc"] = _build()
    return _CACHE["nc"]


def _shard_inputs(x_q, x_kv, Wq, bq, Wkv, bkv, Wo):
    import ml_dtypes

    bf = ml_dtypes.bfloat16
    in_maps = []
    for core in range(NCORES):
        b = core // TPG
        g = core % TPG
        cols = slice(g * CL, (g + 1) * CL)
        in_maps.append(
            {
                "xq": np.ascontiguousarray(x_q[b].astype(bf)),
                "xkv": np.ascontiguousarray(x_kv[b].astype(bf)),
                "wq": np.ascontiguousarray(Wq[:, cols].astype(bf)),
                "wk": np.ascontiguousarray(Wkv[:, :C][:, cols].astype(bf)),
                "wv": np.ascontiguousarray(Wkv[:, C:][:, cols].astype(bf)),
                "wo": np.ascontiguousarray(Wo[g * CL : (g + 1) * CL, :].astype(bf)),
                "bq": np.ascontiguousarray(bq[cols]),
                "bk": np.ascontiguousarray(bkv[:C][cols]),
            }
        )
    return in_maps


def kernel(x_q, x_kv, Wq, bq, Wkv, bkv, Wo, bo):
    from concourse.bass_utils import run_bass_kernel_spmd

    x_q = np.asarray(x_q, dtype=np.float32)
    x_kv = np.asarray(x_kv, dtype=np.float32)
    Wq = np.asarray(Wq, dtype=np.float32)
    bq = np.asarray(bq, dtype=np.float32)
    Wkv = np.asarray(Wkv, dtype=np.float32)
    bkv = np.asarray(bkv, dtype=np.float32)
    Wo = np.asarray(Wo, dtype=np.float32)
    bo = np.asarray(bo, dtype=np.float32)

    nc = _get_nc()
    in_maps = _shard_inputs(x_q, x_kv, Wq, bq, Wkv, bkv, Wo)

    res = run_bass_kernel_spmd(nc, in_maps, core_ids=list(range(NCORES)))

    # host-side gather: sum tensor-parallel partials; add exact bias terms
    bias_full = bkv[C:] @ Wo + bo  # v-bias through Wo, plus output bias
    out = np.zeros((B, T, C), dtype=np.float32)
    for core in range(NCORES):
        out[core // TPG] += res.results[core]["out"]
    out += bias_full[None, None, :]
    return out


# revision 8
# speedup vs baseline: 1.0269x; 1.0030x over previous
"""Cross-attention Bass/Tile kernel for Trainium2, sharded over 8 NeuronCores.

Problem (fixed shapes): B=2, T=2048, C=1024, H=16 heads, D=64.
    q = x_q @ Wq + bq;  kv = x_kv @ Wkv + bkv;  k, v = split(kv)
    y = softmax(q k^T / sqrt(D)) v;  out = y @ Wo + bo
Sharding: 8 cores = 2 (batch) x 4 (head groups of 4 heads, 256 channels).

Fully bf16 dataflow (host casts x and weights; rel-err budget 2e-2 >> bf16
noise; PSUM accumulation stays fp32 except the single-shot S logits which
land in bf16 PSUM to halve bank usage).  Attention AV uses exp(S) as the
matmul *stationary* ([tk,128] x [tk,65] -> [tq,65]) so each product streams
65 moving columns instead of 512 (AV: 131k -> 67k PE cycles).  The softmax
denominator rides as a ones-column of V and lands per-partition; the
normalization is a DVE reciprocal + per-partition scalar multiply.
Normalized y transposes back to y^T with bf16 PE transposes.

Schedule: the two tq=0 attention passes are woven *into* phase A (K/V
prep) so the Activation engine's exp stream starts ~10us in; Q-prep for
tq+1 and the deferred output projection weave into the attention passes
as PE filler.  PSUM: 2 x [128,1024 bf16] S slots (2 banks), 4 x
[128,4,65 f32] y accumulators (4), 2 x 2KB weave slots (2).
"""

import numpy as np

B = 2
T = 2048
C = 1024
H = 16
D = 64
NCORES = 8
TPG = 4  # tensor-parallel group size (head groups)
HL = H // TPG  # heads per core = 4
CL = HL * D  # local channels = 256
P = 128

_CACHE = {}


def _build():
    import concourse.tile as tile
    from concourse import bacc, mybir
    from concourse.masks import make_identity

    f32 = mybir.dt.float32
    bf16 = mybir.dt.bfloat16
    Exp = mybir.ActivationFunctionType.Exp

    nc = bacc.Bacc("TRN2", target_bir_lowering=False, debug=False)

    xq_d = nc.dram_tensor("xq", [T, C], bf16, kind="ExternalInput")
    xkv_d = nc.dram_tensor("xkv", [T, C], bf16, kind="ExternalInput")
    wq_d = nc.dram_tensor("wq", [C, CL], bf16, kind="ExternalInput")
    wk_d = nc.dram_tensor("wk", [C, CL], bf16, kind="ExternalInput")
    wv_d = nc.dram_tensor("wv", [C, CL], bf16, kind="ExternalInput")
    wo_d = nc.dram_tensor("wo", [CL, C], bf16, kind="ExternalInput")
    bq_d = nc.dram_tensor("bq", [CL], f32, kind="ExternalInput")
    bk_d = nc.dram_tensor("bk", [CL], f32, kind="ExternalInput")
    out_d = nc.dram_tensor("out", [T, C], bf16, kind="ExternalOutput")

    KC = C // P  # 8 contraction chunks for the projections
    NT = T // P  # 16 token chunks of 128
    NQ = 4  # tq chunks of 512
    QW = T // NQ  # 512
    DC = CL // P  # 2 chunks of d_local
    LAG = 5

    with tile.TileContext(nc) as tc:
        with (
            tc.tile_pool(name="const", bufs=1) as const,
            tc.tile_pool(name="persist", bufs=1) as persist,
            tc.tile_pool(name="xnat", bufs=4) as xnat,
            tc.tile_pool(name="xt", bufs=1) as xtp,
            tc.tile_pool(name="ework", bufs=54) as ework,
            tc.tile_pool(name="norm2", bufs=2) as norm2,
            tc.tile_pool(name="outst", bufs=6) as outst,
        ):
            # ---- constants / weights (weights via SWDGE, one DMA per
            # tensor, first-consumer first, so HWDGE is free for x loads
            # and nothing stalls on trickled weight chunks) ----
            ident = const.tile([P, P], f32)
            make_identity(nc, ident)
            identb = const.tile([P, P], bf16)
            nc.vector.tensor_copy(identb, ident)
            ones4_f32 = const.tile([P, HL, 1], f32)
            nc.vector.memset(ones4_f32, 1.0)

            wq_sb = const.tile([P, KC, CL], bf16)
            wk_sb = const.tile([P, KC, CL], bf16)
            wv_sb = const.tile([P, KC, CL], bf16)
            wo_sb = const.tile([P, DC, C], bf16)
            for w_sb, w_d in ((wv_sb, wv_d), (wq_sb, wq_d), (wk_sb, wk_d)):
                nc.gpsimd.dma_start(
                    w_sb, w_d.rearrange("(o p) d -> p o d", p=P)
                )
            bq_sb = const.tile([P, DC], f32)
            bk_sb = const.tile([P, DC], f32)

            # ---- persistent activations ----
            qt_sb = persist.tile([P, DC, T], bf16)  # Q^T  [d, t]
            kt_sb = persist.tile([P, DC, T], bf16)  # K^T  [d, t]
            v_sb = persist.tile([P, NT, HL, 66], bf16)  # V|1 [t, h, d+1]
            yt_sb = persist.tile([P, DC, T], bf16)  # y^T  [d, t] (normalized)

            # ---- kernel-wide PSUM ----
            ps_s = tc.alloc_tile_pool(name="ps_s", bufs=2, space="PSUM")
            ps_acc = tc.alloc_tile_pool(name="ps_acc", bufs=1, space="PSUM")
            ps_y = tc.alloc_tile_pool(name="ps_y", bufs=2, space="PSUM")

            # ---------- emission helpers ----------
            def q_prep_units(tq):
                """Work units (thunks) producing xq^T and Q^T for `tq`."""
                xq_t = xtp.tile([P, KC, QW], bf16, tag="xqT", name="xq_t")
                units = []
                trs = []
                state = {}
                for ts_ in range(4):
                    tch = tq * 4 + ts_

                    def dma_u(ts_=ts_, tch=tch):
                        x_nat = xnat.tile([P, C], bf16, tag="xq_nat", name="x_nat")
                        state[ts_] = x_nat
                        nc.sync.dma_start(x_nat, xq_d[tch * P : (tch + 1) * P, :])

                    units.append(dma_u)
                    for grp in range(2):

                        def tr_u(ts_=ts_, grp=grp):
                            x_nat = state[ts_]
                            tp = ps_y.tile([P, 4 * P], bf16, tag="y", name="tp")
                            for cc in range(4):
                                c = grp * 4 + cc
                                nc.tensor.transpose(
                                    tp[:, cc * P : (cc + 1) * P],
                                    x_nat[:, c * P : (c + 1) * P],
                                    identb,
                                )
                            nc.vector.tensor_copy(
                                xq_t[
                                    :, grp * 4 : (grp + 1) * 4, ts_ * P : (ts_ + 1) * P
                                ],
                                tp.rearrange("p (c t) -> p c t", c=4),
                            )

                        trs.append(tr_u)
                units.extend(trs)  # all 4 DMAs go out before any PE work
                for dc in range(DC):

                    def proj_u(dc=dc):
                        pp = ps_y.tile([P, QW], f32, tag="y", name="pp")
                        for c in range(KC):
                            nc.tensor.matmul(
                                pp,
                                wq_sb[:, c, dc * P : (dc + 1) * P],
                                xq_t[:, c, :],
                                start=(c == 0),
                                stop=(c == KC - 1),
                            )
                        nc.vector.tensor_scalar_add(
                            qt_sb[:, dc, tq * QW : (tq + 1) * QW],
                            pp,
                            bq_sb[:, dc : dc + 1],
                        )

                    units.append(proj_u)
                return units

            def po_units(tq, on_act=False):
                """Output-projection work units for `tq` (yt must be final)."""
                units = []
                for ts_ in range(4):
                    tch = tq * 4 + ts_
                    for co in range(2):

                        def u(tch=tch, co=co):
                            po = ps_y.tile([P, QW], f32, tag="y", name="po")
                            for dc in range(DC):
                                nc.tensor.matmul(
                                    po,
                                    yt_sb[:, dc, tch * P : (tch + 1) * P],
                                    wo_sb[:, dc, co * QW : (co + 1) * QW],
                                    start=(dc == 0),
                                    stop=(dc == DC - 1),
                                )
                            o_st = outst.tile([P, QW], bf16, tag="o")
                            if on_act and (ts_ + co) % 2 == 0:
                                nc.scalar.copy(o_st, po)
                            else:
                                nc.vector.tensor_copy(o_st, po)
                            # final batch alternates HWDGE/SWDGE so the two
                            # descriptor generators overlap in the tail
                            dma_q = nc.sync
                            dma_q.dma_start(
                                out_d[
                                    tch * P : (tch + 1) * P, co * QW : (co + 1) * QW
                                ],
                                o_st,
                            )

                        units.append(u)
                return units

            # ---------- attention streaming machinery ----------
            y_tiles = {}
            e_tiles = {}
            yq = []  # FIFO of (k, hc, tk) awaiting their AV matmuls
            unit_q = []  # (tag, thunk) PE filler work units
            s1c = [0, 0]  # held S(1,hc) stream cursors during phase A

            def emit_sexp(k, hc, tk):
                sp = ps_s.tile([P, 2 * QW], f32, tag="s", name="sp")
                for hh in range(2):
                    nc.tensor.matmul(
                        sp[:, hh * QW : (hh + 1) * QW],
                        kt_sb[hh * 64 : (hh + 1) * 64, hc, tk * P : (tk + 1) * P],
                        qt_sb[hh * 64 : (hh + 1) * 64, hc, k * QW : (k + 1) * QW],
                        start=True,
                        stop=True,
                        tile_position=(hh * 64, 0),
                    )
                e2 = ework.tile([P, 2 * QW], bf16, tag="e", name="e2")
                nc.scalar.activation(e2, sp, Exp, scale=0.125)
                e_tiles[(k, hc, tk)] = e2

            def emit_y(k, hc, tk):
                """AV partials: exp(S) chunk as stationary, V|1 as moving."""
                if (k, hc) not in y_tiles:
                    y_tiles[(k, hc)] = [
                        ps_acc.tile([P, 4, 65], f32, tag=f"acc{i}", name=f"y_ps{i}")
                        for i in range(2)
                    ]
                y_pair = y_tiles[(k, hc)]
                e2 = e_tiles.pop((k, hc, tk))
                for hh in range(2):
                    h = 2 * hc + hh
                    for cq in range(4):
                        # one accumulation group per PSUM bank (= per hh
                        # tile): start zeroes the whole 2KB zero-region, so
                        # only the very first matmul into the bank starts
                        # and only the very last stops
                        nc.tensor.matmul(
                            y_pair[hh][:, cq, :],
                            e2[:, hh * QW + cq * P : hh * QW + (cq + 1) * P],
                            v_sb[:, tk, h, 0:65],
                            start=(tk == 0 and cq == 0),
                            stop=(tk == NT - 1 and cq == 3),
                        )

            def emit_norm(k, hc, fuse_po=False):
                """Normalize by the ridden-along denominator; build y^T.
                With fuse_po (final pass), each 128-token chunk's output
                projection is emitted the moment its y^T slice lands."""
                y_pair = y_tiles.pop((k, hc))
                den = norm2.tile([P, 2, 4], f32, tag="den")
                for hh in range(2):
                    nc.vector.tensor_copy(den[:, hh, :], y_pair[hh][:, :, 64])
                rec = norm2.tile([P, 2, 4], f32, tag="rec")
                with nc.allow_low_precision(reason="softmax denom reciprocal"):
                    nc.vector.reciprocal(rec, den)
                y2 = norm2.tile([P, 4, P], bf16, tag="y2")
                for hh in range(2):
                    for cq in range(4):
                        nc.vector.tensor_scalar_mul(
                            y2[:, cq, hh * 64 : (hh + 1) * 64],
                            y_pair[hh][:, cq, 0:64],
                            rec[:, hh, cq : cq + 1],
                        )
                tp_y = ps_y.tile([P, 4, P], bf16, tag="y", name="tp_y")
                for cq in range(4):
                    nc.tensor.transpose(tp_y[:, cq, :], y2[:, cq, :], identb)
                nc.vector.tensor_copy(
                    yt_sb[:, hc, k * QW : (k + 1) * QW],
                    tp_y.rearrange("p c t -> p (c t)"),
                )
                if hc == DC - 1 and k < NQ - 1:
                    unit_q.extend(("po", u) for u in po_units(k))

            def attn_step(k, hc, tk, hold=False, lag=LAG):
                """Stream one S/exp step.  hold=True defers the AV work
                entirely (retired later via push_retirement units)."""
                emit_sexp(k, hc, tk)
                if not hold:
                    yq.append((k, hc, tk))
                    if len(yq) > lag:
                        emit_y(*yq.pop(0))

            def push_retirement(k, hc):
                """Queue a held pass's AV matmuls + normalization as filler
                units; they pop during the next pass's S stream."""
                for tk in range(NT):
                    unit_q.append(
                        ("ret", lambda k=k, hc=hc, tk=tk: emit_y(k, hc, tk))
                    )
                unit_q.append(("ret", lambda k=k, hc=hc: emit_norm(k, hc)))

            def pop_units(n):
                for _ in range(n):
                    if unit_q:
                        unit_q.pop(0)[1]()

            # ---- phase A: K^T and V prep with the two tq=0 attention
            # passes woven in (exp starts as soon as the first K half and
            # Q^T(tq0) exist) ----
            # first two x_kv chunk DMAs go out before the x_q block so the
            # PE has transpose work at t=0
            kv_pre = []
            for i in range(2):
                t = xnat.tile([P, C], bf16, tag="xkv_nat", name="kv_nat")
                nc.scalar.dma_start(t, xkv_d[i * P : (i + 1) * P, :])
                kv_pre.append(t)
            qunits = q_prep_units(0)
            for u in qunits[:4]:  # xq DMAs up front
                u()
            qrest = qunits[4:]
            # remaining weights after the x loads: their DMA transfers
            # yield the shared DMA engines to the latency-critical x_q path
            nc.gpsimd.dma_start(bq_sb, bq_d.rearrange("(o p) -> p o", p=P))
            nc.gpsimd.dma_start(bk_sb, bk_d.rearrange("(o p) -> p o", p=P))
            nc.gpsimd.dma_start(wo_sb, wo_d.rearrange("(o p) n -> p o n", p=P))
            for tq in range(NQ):
                xkv_t = xtp.tile([P, KC, QW], bf16, tag="xkvT", name="xkv_t")
                for half in range(2):
                    # transposes + xkv^T copies for the half's two chunks:
                    # nothing else enters the DVE queue ahead of the
                    # latency-critical xkv^T -> K-proj -> kt-bias chain
                    for s2 in range(2):
                        ts_ = half * 2 + s2
                        tch = tq * 4 + ts_
                        if tch < len(kv_pre):
                            kv_nat = kv_pre[tch]
                        else:
                            kv_nat = xnat.tile(
                                [P, C], bf16, tag="xkv_nat", name="kv_nat"
                            )
                            nc.sync.dma_start(
                                kv_nat, xkv_d[tch * P : (tch + 1) * P, :]
                            )
                        tp = ps_y.tile([P, KC * P], bf16, tag="y", name="tp8")
                        for c in range(KC):
                            nc.tensor.transpose(
                                tp[:, c * P : (c + 1) * P],
                                kv_nat[:, c * P : (c + 1) * P],
                                identb,
                            )
                        nc.vector.tensor_copy(
                            xkv_t[:, :, ts_ * P : (ts_ + 1) * P],
                            tp.rearrange("p (c t) -> p c t", c=KC),
                        )
                    # finish Q-prep(0) before the first K half completes
                    while qrest:
                        qrest.pop(0)()
                    # queue Q-prep(1) early; it must finish before the held
                    # S(1,*) streams start at tq==2
                    if tq == 0 and half == 0:
                        unit_q.extend(("qp1", u) for u in q_prep_units(1))
                    hsl = slice(half * 256, (half + 1) * 256)
                    for dc in range(DC):
                        pp = ps_y.tile([P, 256], f32, tag="y", name="ppk")
                        for c in range(KC):
                            nc.tensor.matmul(
                                pp,
                                wk_sb[:, c, dc * P : (dc + 1) * P],
                                xkv_t[:, c, hsl],
                                start=(c == 0),
                                stop=(c == KC - 1),
                            )
                        nc.vector.tensor_scalar_add(
                            kt_sb[:, dc, tq * QW + half * 256 : tq * QW + (half + 1) * 256],
                            pp,
                            bk_sb[:, dc : dc + 1],
                        )
                    if tq == 2 and half == 0:
                        # correctness: Q^T(1) writes must be emitted before
                        # the held S(1,*) streams read them
                        while any(t == "qp1" for t, _ in unit_q):
                            pop_units(1)
                    # attention on the two freshly available tk chunks.
                    # Pass (0,0) runs with live (lagged) AV; (0,1) and,
                    # once Q^T(1) exists, (1,0)/(1,1) stream S/exp held
                    # so the ACT exp pipeline is dense through phase A.
                    for tkn in (
                        tq * 4 + half * 2,
                        tq * 4 + half * 2 + 1,
                    ):
                        attn_step(0, 0, tkn)
                    for tkn in (
                        tq * 4 + half * 2,
                        tq * 4 + half * 2 + 1,
                    ):
                        attn_step(0, 1, tkn, hold=True)
                    if tq >= 2:
                        for hcx in range(DC):
                            for _ in range(2):
                                if s1c[hcx] < NT - 8:
                                    attn_step(1, hcx, s1c[hcx], hold=True)
                                    s1c[hcx] += 1
                    # deferred V projections + copies for the half's chunks
                    for s2 in range(2):
                        ts_ = half * 2 + s2
                        tch = tq * 4 + ts_
                        pv = ps_y.tile([P, CL], f32, tag="y", name="pv")
                        for c in range(KC):
                            nc.tensor.matmul(
                                pv,
                                xkv_t[:, c, ts_ * P : (ts_ + 1) * P],
                                wv_sb[:, c, :],
                                start=(c == 0),
                                stop=(c == KC - 1),
                            )
                        nc.vector.tensor_copy(
                            v_sb[:, tch, :, 0:64],
                            pv.rearrange("p (h d) -> p h d", h=HL),
                        )
                        nc.vector.tensor_copy(v_sb[:, tch, :, 64:65], ones4_f32)
                    pop_units(1)

            # finish the held S(1,*) streams, retire the phase-A passes
            unit_q.extend(("qp2", u) for u in q_prep_units(2))
            for hcx in range(DC):
                for tkx in range(s1c[hcx], NT):
                    attn_step(1, hcx, tkx, hold=True)
                    pop_units(1)
            while yq:
                emit_y(*yq.pop(0))
            emit_norm(0, 0)
            push_retirement(0, 1)
            push_retirement(1, 0)
            push_retirement(1, 1)

            # ---- phase B: remaining S/exp streams; AV+norm of each pass
            # retires as filler during the next pass's stream ----
            for k, hc in [(2, 0), (2, 1), (3, 0), (3, 1)]:
                last = k == NQ - 1 and hc == DC - 1
                if hc == 0:
                    # Q-prep(k) must be fully emitted before S reads Q^T(k)
                    while any(t == f"qp{k}" for t, _ in unit_q):
                        pop_units(1)
                    if k + 1 < NQ:
                        unit_q.extend(
                            (f"qp{k + 1}", u) for u in q_prep_units(k + 1)
                        )
                for tk in range(NT):
                    attn_step(k, hc, tk, hold=True)
                    pop_units(3 if len(unit_q) > 30 else 2)
                if not last:
                    push_retirement(k, hc)
            while unit_q:
                pop_units(1)
            for tk in range(NT):
                emit_y(NQ - 1, DC - 1, tk)
            emit_norm(NQ - 1, DC - 1)
            for u in po_units(NQ - 1, on_act=True):
                u()
            while unit_q:
                pop_units(1)

            ps_y.release()
            ps_acc.release()
            ps_s.release()

    nc.compile()
    return nc


def _get_nc():
    if "nc" not in _CACHE:
        _CACHE["nc"] = _build()
    return _CACHE["nc"]


def _shard_inputs(x_q, x_kv, Wq, bq, Wkv, bkv, Wo):
    import ml_dtypes

    bf = ml_dtypes.bfloat16
    in_maps = []
    for core in range(NCORES):
        b = core // TPG
        g = core % TPG
        cols = slice(g * CL, (g + 1) * CL)
        in_maps.append(
            {
                "xq": np.ascontiguousarray(x_q[b].astype(bf)),
                "xkv": np.ascontiguousarray(x_kv[b].astype(bf)),
                "wq": np.ascontiguousarray(Wq[:, cols].astype(bf)),
                "wk": np.ascontiguousarray(Wkv[:, :C][:, cols].astype(bf)),
                "wv": np.ascontiguousarray(Wkv[:, C:][:, cols].astype(bf)),
                "wo": np.ascontiguousarray(Wo[g * CL : (g + 1) * CL, :].astype(bf)),
                "bq": np.ascontiguousarray(bq[cols]),
                "bk": np.ascontiguousarray(bkv[:C][cols]),
            }
        )
    return in_maps


def kernel(x_q, x_kv, Wq, bq, Wkv, bkv, Wo, bo):
    from concourse.bass_utils import run_bass_kernel_spmd

    x_q = np.asarray(x_q, dtype=np.float32)
    x_kv = np.asarray(x_kv, dtype=np.float32)
    Wq = np.asarray(Wq, dtype=np.float32)
    bq = np.asarray(bq, dtype=np.float32)
    Wkv = np.asarray(Wkv, dtype=np.float32)
    bkv = np.asarray(bkv, dtype=np.float32)
    Wo = np.asarray(Wo, dtype=np.float32)
    bo = np.asarray(bo, dtype=np.float32)

    nc = _get_nc()
    in_maps = _shard_inputs(x_q, x_kv, Wq, bq, Wkv, bkv, Wo)

    res = run_bass_kernel_spmd(nc, in_maps, core_ids=list(range(NCORES)))

    # host-side gather: sum tensor-parallel partials; add exact bias terms
    bias_full = bkv[C:] @ Wo + bo  # v-bias through Wo, plus output bias
    out = np.zeros((B, T, C), dtype=np.float32)
    for core in range(NCORES):
        out[core // TPG] += np.asarray(res.results[core]["out"], dtype=np.float32)
    out += bias_full[None, None, :]
    return out


# revision 9
# speedup vs baseline: 1.0300x; 1.0030x over previous
"""Cross-attention Bass/Tile kernel for Trainium2, sharded over 8 NeuronCores.

Problem (fixed shapes): B=2, T=2048, C=1024, H=16 heads, D=64.
    q = x_q @ Wq + bq;  kv = x_kv @ Wkv + bkv;  k, v = split(kv)
    y = softmax(q k^T / sqrt(D)) v;  out = y @ Wo + bo
Sharding: 8 cores = 2 (batch) x 4 (head groups of 4 heads, 256 channels).

Fully bf16 dataflow (host casts x and weights; rel-err budget 2e-2 >> bf16
noise; PSUM accumulation stays fp32 except the single-shot S logits which
land in bf16 PSUM to halve bank usage).  Attention AV uses exp(S) as the
matmul *stationary* ([tk,128] x [tk,65] -> [tq,65]) so each product streams
65 moving columns instead of 512 (AV: 131k -> 67k PE cycles).  The softmax
denominator rides as a ones-column of V and lands per-partition; the
normalization is a DVE reciprocal + per-partition scalar multiply.
Normalized y transposes back to y^T with bf16 PE transposes.

Schedule: the two tq=0 attention passes are woven *into* phase A (K/V
prep) so the Activation engine's exp stream starts ~10us in; Q-prep for
tq+1 and the deferred output projection weave into the attention passes
as PE filler.  PSUM: 2 x [128,1024 bf16] S slots (2 banks), 4 x
[128,4,65 f32] y accumulators (4), 2 x 2KB weave slots (2).
"""

import numpy as np

B = 2
T = 2048
C = 1024
H = 16
D = 64
NCORES = 8
TPG = 4  # tensor-parallel group size (head groups)
HL = H // TPG  # heads per core = 4
CL = HL * D  # local channels = 256
P = 128

_CACHE = {}


def _build():
    import concourse.tile as tile
    from concourse import bacc, mybir
    from concourse.masks import make_identity

    f32 = mybir.dt.float32
    bf16 = mybir.dt.bfloat16
    Exp = mybir.ActivationFunctionType.Exp

    nc = bacc.Bacc("TRN2", target_bir_lowering=False, debug=False)

    xq_d = nc.dram_tensor("xq", [T, C], bf16, kind="ExternalInput")
    xkv_d = nc.dram_tensor("xkv", [T, C], bf16, kind="ExternalInput")
    wq_d = nc.dram_tensor("wq", [C, CL], bf16, kind="ExternalInput")
    wk_d = nc.dram_tensor("wk", [C, CL], bf16, kind="ExternalInput")
    wv_d = nc.dram_tensor("wv", [C, CL], bf16, kind="ExternalInput")
    wo_d = nc.dram_tensor("wo", [CL, C], bf16, kind="ExternalInput")
    bq_d = nc.dram_tensor("bq", [CL], f32, kind="ExternalInput")
    bk_d = nc.dram_tensor("bk", [CL], f32, kind="ExternalInput")
    out_d = nc.dram_tensor("out", [T, C], bf16, kind="ExternalOutput")

    KC = C // P  # 8 contraction chunks for the projections
    NT = T // P  # 16 token chunks of 128
    NQ = 4  # tq chunks of 512
    QW = T // NQ  # 512
    DC = CL // P  # 2 chunks of d_local
    LAG = 5

    with tile.TileContext(nc) as tc:
        with (
            tc.tile_pool(name="const", bufs=1) as const,
            tc.tile_pool(name="persist", bufs=1) as persist,
            tc.tile_pool(name="xnat", bufs=4) as xnat,
            tc.tile_pool(name="xt", bufs=1) as xtp,
            tc.tile_pool(name="ework", bufs=54) as ework,
            tc.tile_pool(name="norm2", bufs=2) as norm2,
            tc.tile_pool(name="outst", bufs=6) as outst,
        ):
            # ---- constants / weights (weights via SWDGE, one DMA per
            # tensor, first-consumer first, so HWDGE is free for x loads
            # and nothing stalls on trickled weight chunks) ----
            ident = const.tile([P, P], f32)
            make_identity(nc, ident)
            identb = const.tile([P, P], bf16)
            nc.vector.tensor_copy(identb, ident)
            # p-state warmup: keep the PE busy with reader-free identity
            # transposes until the first x chunk lands, so the 3us clock
            # ramp (0.65 -> 2.4 GHz) finishes before real work starts

            ones4_f32 = const.tile([P, HL, 1], f32)
            nc.vector.memset(ones4_f32, 1.0)

            wq_sb = const.tile([P, KC, CL], bf16)
            wk_sb = const.tile([P, KC, CL], bf16)
            wv_sb = const.tile([P, KC, CL], bf16)
            wo_sb = const.tile([P, DC, C], bf16)
            for w_sb, w_d in ((wv_sb, wv_d), (wq_sb, wq_d), (wk_sb, wk_d)):
                nc.gpsimd.dma_start(
                    w_sb, w_d.rearrange("(o p) d -> p o d", p=P)
                )
            bq_sb = const.tile([P, DC], f32)
            bk_sb = const.tile([P, DC], f32)

            # ---- persistent activations ----
            qt_sb = persist.tile([P, DC, T], bf16)  # Q^T  [d, t]
            kt_sb = persist.tile([P, DC, T], bf16)  # K^T  [d, t]
            v_sb = persist.tile([P, NT, HL, 66], bf16)  # V|1 [t, h, d+1]
            yt_sb = persist.tile([P, DC, T], bf16)  # y^T  [d, t] (normalized)

            # ---- kernel-wide PSUM ----
            ps_s = tc.alloc_tile_pool(name="ps_s", bufs=2, space="PSUM")
            ps_acc = tc.alloc_tile_pool(name="ps_acc", bufs=1, space="PSUM")
            ps_y = tc.alloc_tile_pool(name="ps_y", bufs=2, space="PSUM")

            # ---------- emission helpers ----------
            def q_prep_units(tq):
                """Work units (thunks) producing xq^T and Q^T for `tq`."""
                xq_t = xtp.tile([P, KC, QW], bf16, tag="xqT", name="xq_t")
                units = []
                trs = []
                state = {}
                for ts_ in range(4):
                    tch = tq * 4 + ts_

                    def dma_u(ts_=ts_, tch=tch):
                        x_nat = xnat.tile([P, C], bf16, tag="xq_nat", name="x_nat")
                        state[ts_] = x_nat
                        nc.sync.dma_start(x_nat, xq_d[tch * P : (tch + 1) * P, :])

                    units.append(dma_u)
                    for grp in range(2):

                        def tr_u(ts_=ts_, grp=grp):
                            x_nat = state[ts_]
                            tp = ps_y.tile([P, 4 * P], bf16, tag="y", name="tp")
                            for cc in range(4):
                                c = grp * 4 + cc
                                nc.tensor.transpose(
                                    tp[:, cc * P : (cc + 1) * P],
                                    x_nat[:, c * P : (c + 1) * P],
                                    identb,
                                )
                            nc.vector.tensor_copy(
                                xq_t[
                                    :, grp * 4 : (grp + 1) * 4, ts_ * P : (ts_ + 1) * P
                                ],
                                tp.rearrange("p (c t) -> p c t", c=4),
                            )

                        trs.append(tr_u)
                units.extend(trs)  # all 4 DMAs go out before any PE work
                for dc in range(DC):

                    def proj_u(dc=dc):
                        pp = ps_y.tile([P, QW], f32, tag="y", name="pp")
                        for c in range(KC):
                            nc.tensor.matmul(
                                pp,
                                wq_sb[:, c, dc * P : (dc + 1) * P],
                                xq_t[:, c, :],
                                start=(c == 0),
                                stop=(c == KC - 1),
                            )
                        nc.vector.tensor_scalar_add(
                            qt_sb[:, dc, tq * QW : (tq + 1) * QW],
                            pp,
                            bq_sb[:, dc : dc + 1],
                        )

                    units.append(proj_u)
                return units

            def po_units(tq, on_act=False):
                """Output-projection work units for `tq` (yt must be final)."""
                units = []
                for ts_ in range(4):
                    tch = tq * 4 + ts_
                    for co in range(2):

                        def u(tch=tch, co=co):
                            po = ps_y.tile([P, QW], f32, tag="y", name="po")
                            for dc in range(DC):
                                nc.tensor.matmul(
                                    po,
                                    yt_sb[:, dc, tch * P : (tch + 1) * P],
                                    wo_sb[:, dc, co * QW : (co + 1) * QW],
                                    start=(dc == 0),
                                    stop=(dc == DC - 1),
                                )
                            o_st = outst.tile([P, QW], bf16, tag="o")
                            if on_act and (ts_ + co) % 2 == 0:
                                nc.scalar.copy(o_st, po)
                            else:
                                nc.vector.tensor_copy(o_st, po)
                            # final batch alternates HWDGE/SWDGE so the two
                            # descriptor generators overlap in the tail
                            dma_q = nc.sync
                            dma_q.dma_start(
                                out_d[
                                    tch * P : (tch + 1) * P, co * QW : (co + 1) * QW
                                ],
                                o_st,
                            )

                        units.append(u)
                return units

            # ---------- attention streaming machinery ----------
            y_tiles = {}
            e_tiles = {}
            yq = []  # FIFO of (k, hc, tk) awaiting their AV matmuls
            unit_q = []  # (tag, thunk) PE filler work units
            s1c = [0, 0]  # held S(1,hc) stream cursors during phase A

            def emit_sexp(k, hc, tk):
                sp = ps_s.tile([P, 2 * QW], f32, tag="s", name="sp")
                for hh in range(2):
                    nc.tensor.matmul(
                        sp[:, hh * QW : (hh + 1) * QW],
                        kt_sb[hh * 64 : (hh + 1) * 64, hc, tk * P : (tk + 1) * P],
                        qt_sb[hh * 64 : (hh + 1) * 64, hc, k * QW : (k + 1) * QW],
                        start=True,
                        stop=True,
                        tile_position=(hh * 64, 0),
                    )
                e2 = ework.tile([P, 2 * QW], bf16, tag="e", name="e2")
                nc.scalar.activation(e2, sp, Exp, scale=0.125)
                e_tiles[(k, hc, tk)] = e2

            def emit_y(k, hc, tk):
                """AV partials: exp(S) chunk as stationary, V|1 as moving."""
                if (k, hc) not in y_tiles:
                    y_tiles[(k, hc)] = [
                        ps_acc.tile([P, 4, 65], f32, tag=f"acc{i}", name=f"y_ps{i}")
                        for i in range(2)
                    ]
                y_pair = y_tiles[(k, hc)]
                e2 = e_tiles.pop((k, hc, tk))
                for hh in range(2):
                    h = 2 * hc + hh
                    for cq in range(4):
                        # one accumulation group per PSUM bank (= per hh
                        # tile): start zeroes the whole 2KB zero-region, so
                        # only the very first matmul into the bank starts
                        # and only the very last stops
                        nc.tensor.matmul(
                            y_pair[hh][:, cq, :],
                            e2[:, hh * QW + cq * P : hh * QW + (cq + 1) * P],
                            v_sb[:, tk, h, 0:65],
                            start=(tk == 0 and cq == 0),
                            stop=(tk == NT - 1 and cq == 3),
                        )

            def emit_norm(k, hc, fuse_po=False):
                """Normalize by the ridden-along denominator; build y^T.
                With fuse_po (final pass), each 128-token chunk's output
                projection is emitted the moment its y^T slice lands."""
                y_pair = y_tiles.pop((k, hc))
                den = norm2.tile([P, 2, 4], f32, tag="den")
                for hh in range(2):
                    nc.vector.tensor_copy(den[:, hh, :], y_pair[hh][:, :, 64])
                rec = norm2.tile([P, 2, 4], f32, tag="rec")
                with nc.allow_low_precision(reason="softmax denom reciprocal"):
                    nc.vector.reciprocal(rec, den)
                y2 = norm2.tile([P, 4, P], bf16, tag="y2")
                for hh in range(2):
                    for cq in range(4):
                        nc.vector.tensor_scalar_mul(
                            y2[:, cq, hh * 64 : (hh + 1) * 64],
                            y_pair[hh][:, cq, 0:64],
                            rec[:, hh, cq : cq + 1],
                        )
                tp_y = ps_y.tile([P, 4, P], bf16, tag="y", name="tp_y")
                for cq in range(4):
                    nc.tensor.transpose(tp_y[:, cq, :], y2[:, cq, :], identb)
                nc.vector.tensor_copy(
                    yt_sb[:, hc, k * QW : (k + 1) * QW],
                    tp_y.rearrange("p c t -> p (c t)"),
                )
                if hc == DC - 1 and k < NQ - 1:
                    unit_q.extend(("po", u) for u in po_units(k))

            def attn_step(k, hc, tk, hold=False, lag=LAG):
                """Stream one S/exp step.  hold=True defers the AV work
                entirely (retired later via push_retirement units)."""
                emit_sexp(k, hc, tk)
                if not hold:
                    yq.append((k, hc, tk))
                    if len(yq) > lag:
                        emit_y(*yq.pop(0))

            def push_retirement(k, hc):
                """Queue a held pass's AV matmuls + normalization as filler
                units; they pop during the next pass's S stream."""
                for tk in range(NT):
                    unit_q.append(
                        ("ret", lambda k=k, hc=hc, tk=tk: emit_y(k, hc, tk))
                    )
                unit_q.append(("ret", lambda k=k, hc=hc: emit_norm(k, hc)))

            def pop_units(n):
                for _ in range(n):
                    if unit_q:
                        unit_q.pop(0)[1]()

            # ---- phase A: K^T and V prep with the two tq=0 attention
            # passes woven in (exp starts as soon as the first K half and
            # Q^T(tq0) exist) ----
            # first two x_kv chunk DMAs go out before the x_q block so the
            # PE has transpose work at t=0
            wtile = ps_y.tile([P, 4 * P], bf16, tag="y", name="warm")
            for i in range(24):
                nc.tensor.transpose(
                    wtile[:, (i % 4) * P : (i % 4 + 1) * P], identb, identb
                )
            kv_pre = []
            for i in range(2):
                t = xnat.tile([P, C], bf16, tag="xkv_nat", name="kv_nat")
                nc.scalar.dma_start(t, xkv_d[i * P : (i + 1) * P, :])
                kv_pre.append(t)
            qunits = q_prep_units(0)
            for u in qunits[:4]:  # xq DMAs up front
                u()
            qrest = qunits[4:]
            # remaining weights after the x loads: their DMA transfers
            # yield the shared DMA engines to the latency-critical x_q path
            nc.gpsimd.dma_start(bq_sb, bq_d.rearrange("(o p) -> p o", p=P))
            nc.gpsimd.dma_start(bk_sb, bk_d.rearrange("(o p) -> p o", p=P))
            nc.gpsimd.dma_start(wo_sb, wo_d.rearrange("(o p) n -> p o n", p=P))
            for tq in range(NQ):
                xkv_t = xtp.tile([P, KC, QW], bf16, tag="xkvT", name="xkv_t")
                for half in range(2):
                    # transposes + xkv^T copies for the half's two chunks:
                    # nothing else enters the DVE queue ahead of the
                    # latency-critical xkv^T -> K-proj -> kt-bias chain
                    for s2 in range(2):
                        ts_ = half * 2 + s2
                        tch = tq * 4 + ts_
                        if tch < len(kv_pre):
                            kv_nat = kv_pre[tch]
                        else:
                            kv_nat = xnat.tile(
                                [P, C], bf16, tag="xkv_nat", name="kv_nat"
                            )
                            nc.sync.dma_start(
                                kv_nat, xkv_d[tch * P : (tch + 1) * P, :]
                            )
                        tp = ps_y.tile([P, KC * P], bf16, tag="y", name="tp8")
                        for c in range(KC):
                            nc.tensor.transpose(
                                tp[:, c * P : (c + 1) * P],
                                kv_nat[:, c * P : (c + 1) * P],
                                identb,
                            )
                        nc.vector.tensor_copy(
                            xkv_t[:, :, ts_ * P : (ts_ + 1) * P],
                            tp.rearrange("p (c t) -> p c t", c=KC),
                        )
                    # finish Q-prep(0) before the first K half completes
                    while qrest:
                        qrest.pop(0)()
                    # queue Q-prep(1) early; it must finish before the held
                    # S(1,*) streams start at tq==2
                    if tq == 0 and half == 0:
                        unit_q.extend(("qp1", u) for u in q_prep_units(1))
                    hsl = slice(half * 256, (half + 1) * 256)
                    for dc in range(DC):
                        pp = ps_y.tile([P, 256], f32, tag="y", name="ppk")
                        for c in range(KC):
                            nc.tensor.matmul(
                                pp,
                                wk_sb[:, c, dc * P : (dc + 1) * P],
                                xkv_t[:, c, hsl],
                                start=(c == 0),
                                stop=(c == KC - 1),
                            )
                        nc.vector.tensor_scalar_add(
                            kt_sb[:, dc, tq * QW + half * 256 : tq * QW + (half + 1) * 256],
                            pp,
                            bk_sb[:, dc : dc + 1],
                        )
                    if tq == 2 and half == 0:
                        # correctness: Q^T(1) writes must be emitted before
                        # the held S(1,*) streams read them
                        while any(t == "qp1" for t, _ in unit_q):
                            pop_units(1)
                    # attention on the two freshly available tk chunks.
                    # Pass (0,0) runs with live (lagged) AV; (0,1) and,
                    # once Q^T(1) exists, (1,0)/(1,1) stream S/exp held
                    # so the ACT exp pipeline is dense through phase A.
                    for tkn in (
                        tq * 4 + half * 2,
                        tq * 4 + half * 2 + 1,
                    ):
                        attn_step(0, 0, tkn)
                    for tkn in (
                        tq * 4 + half * 2,
                        tq * 4 + half * 2 + 1,
                    ):
                        attn_step(0, 1, tkn, hold=True)
                    if tq >= 2:
                        for hcx in range(DC):
                            for _ in range(2):
                                if s1c[hcx] < NT - 8:
                                    attn_step(1, hcx, s1c[hcx], hold=True)
                                    s1c[hcx] += 1
                    # deferred V projections + copies for the half's chunks
                    for s2 in range(2):
                        ts_ = half * 2 + s2
                        tch = tq * 4 + ts_
                        pv = ps_y.tile([P, CL], f32, tag="y", name="pv")
                        for c in range(KC):
                            nc.tensor.matmul(
                                pv,
                                xkv_t[:, c, ts_ * P : (ts_ + 1) * P],
                                wv_sb[:, c, :],
                                start=(c == 0),
                                stop=(c == KC - 1),
                            )
                        nc.vector.tensor_copy(
                            v_sb[:, tch, :, 0:64],
                            pv.rearrange("p (h d) -> p h d", h=HL),
                        )
                        nc.vector.tensor_copy(v_sb[:, tch, :, 64:65], ones4_f32)
                    pop_units(1)

            # finish the held S(1,*) streams, retire the phase-A passes
            unit_q.extend(("qp2", u) for u in q_prep_units(2))
            for hcx in range(DC):
                for tkx in range(s1c[hcx], NT):
                    attn_step(1, hcx, tkx, hold=True)
                    pop_units(1)
            while yq:
                emit_y(*yq.pop(0))
            emit_norm(0, 0)
            push_retirement(0, 1)
            push_retirement(1, 0)
            push_retirement(1, 1)

            # ---- phase B: remaining S/exp streams; AV+norm of each pass
            # retires as filler during the next pass's stream ----
            for k, hc in [(2, 0), (2, 1), (3, 0), (3, 1)]:
                last = k == NQ - 1 and hc == DC - 1
                if hc == 0:
                    # Q-prep(k) must be fully emitted before S reads Q^T(k)
                    while any(t == f"qp{k}" for t, _ in unit_q):
                        pop_units(1)
                    if k + 1 < NQ:
                        unit_q.extend(
                            (f"qp{k + 1}", u) for u in q_prep_units(k + 1)
                        )
                for tk in range(NT):
                    attn_step(k, hc, tk, hold=True)
                    pop_units(3 if len(unit_q) > 30 else 2)
                if not last:
                    push_retirement(k, hc)
            while unit_q:
                pop_units(1)
            for tk in range(NT):
                emit_y(NQ - 1, DC - 1, tk)
            emit_norm(NQ - 1, DC - 1)
            for u in po_units(NQ - 1, on_act=True):
                u()
            while unit_q:
                pop_units(1)

            ps_y.release()
            ps_acc.release()
            ps_s.release()

    nc.compile()
    return nc


def _get_nc():
    if "nc" not in _CACHE:
        _CACHE["nc"] = _build()
    return _CACHE["nc"]


def _shard_inputs(x_q, x_kv, Wq, bq, Wkv, bkv, Wo):
    import ml_dtypes

    bf = ml_dtypes.bfloat16
    in_maps = []
    for core in range(NCORES):
        b = core // TPG
        g = core % TPG
        cols = slice(g * CL, (g + 1) * CL)
        in_maps.append(
            {
                "xq": np.ascontiguousarray(x_q[b].astype(bf)),
                "xkv": np.ascontiguousarray(x_kv[b].astype(bf)),
                "wq": np.ascontiguousarray(Wq[:, cols].astype(bf)),
                "wk": np.ascontiguousarray(Wkv[:, :C][:, cols].astype(bf)),
                "wv": np.ascontiguousarray(Wkv[:, C:][:, cols].astype(bf)),
                "wo": np.ascontiguousarray(Wo[g * CL : (g + 1) * CL, :].astype(bf)),
                "bq": np.ascontiguousarray(bq[cols]),
                "bk": np.ascontiguousarray(bkv[:C][cols]),
            }
        )
    return in_maps


def kernel(x_q, x_kv, Wq, bq, Wkv, bkv, Wo, bo):
    from concourse.bass_utils import run_bass_kernel_spmd

    x_q = np.asarray(x_q, dtype=np.float32)
    x_kv = np.asarray(x_kv, dtype=np.float32)
    Wq = np.asarray(Wq, dtype=np.float32)
    bq = np.asarray(bq, dtype=np.float32)
    Wkv = np.asarray(Wkv, dtype=np.float32)
    bkv = np.asarray(bkv, dtype=np.float32)
    Wo = np.asarray(Wo, dtype=np.float32)
    bo = np.asarray(bo, dtype=np.float32)

    nc = _get_nc()
    in_maps = _shard_inputs(x_q, x_kv, Wq, bq, Wkv, bkv, Wo)

    res = run_bass_kernel_spmd(nc, in_maps, core_ids=list(range(NCORES)))

    # host-side gather: sum tensor-parallel partials; add exact bias terms
    bias_full = bkv[C:] @ Wo + bo  # v-bias through Wo, plus output bias
    out = np.zeros((B, T, C), dtype=np.float32)
    for core in range(NCORES):
        out[core // TPG] += np.asarray(res.results[core]["out"], dtype=np.float32)
    out += bias_full[None, None, :]
    return out
